# revision 31
# baseline (speedup 1.0000x reference)
"""Sparse cross-attention (squared-ReLU normalizer) on 8 TRN2 NeuronCores.

Sharding: 8 cores = batch(2) x head-group(4). Each core owns one batch and
4 of 16 heads (a 256-wide slice of hsize): Wq/Wkv column-parallel,
Wo row-parallel (partial outputs summed on host), mask replicated per
batch shard.

Per-core kernel, engine-balanced (bf16 matmuls, fp32 PSUM):
  stage A: rqT (hs,q), rkT (hs,s) via weight-stationary projections into
    [128,1024] PSUM tiles; rv (s, hs+ones) with kT chunks as weights.
    rq evicted on ACT (scale 1/sqrt(a) folded), rk/rv evicted on DVE.
    Input DMA issues are spread across engine queues (weights on DVE,
    mask on ACT, x on sync) so transfers overlap from t=0.
  middle, per unit = (q-tile 512, head): AV matmuls of the previous head
    run first (their t is ready - a contiguous PE burst), then one out-
    projection chunk, then 8 score-matmul pairs -> [128,1024] PSUM, relu
    +nbias on ACT -> u; square IN PLACE (u*=u) and mask mul (t=u*maskT)
    on DVE with the tail s-chunks on GpSimd. AV uses rv chunks as
    weights giving oT (64,q) + denominator row; 1/den via ACT exp(-ln);
    GpSimd partition-broadcast spreads rec; DVE normalizes+evicts oT.
  out projection: oT^T @ woT in PSUM, evicted bf16 (ACT/DVE alternating),
    DMA'd to DRAM; host sums the 4 row-parallel partials in fp32.
"""

import numpy as np
import ml_dtypes

BF16 = ml_dtypes.bfloat16

B, Q, S, D = 2, 2048, 2048, 1024
NUM_HEAD, ADIM = 16, 64
HSIZE = NUM_HEAD * ADIM
N_CORES = 8
GROUPS = 4                  # head groups (tensor-parallel dim)
HPG = NUM_HEAD // GROUPS    # 4 heads per core
HS = HPG * ADIM             # 256: per-core hsize slice
P = 128

POOL_SC = 3                 # trailing s-chunks per mul pass on GpSimd

_COMPILED = None


def _build(q=Q, s=S, d=D, hpg=HPG, adim=ADIM, qt=512):
    """Build + compile the per-core Bass program. Returns the Bacc."""
    from contextlib import ExitStack
    import concourse.bass as bass
    import concourse.mybir as mybir
    import concourse.tile as tile
    from concourse import bacc

    fp32 = mybir.dt.float32
    bf16 = mybir.dt.bfloat16
    AF = mybir.ActivationFunctionType

    hs = hpg * adim
    DC = d // P          # contraction chunks for projections (8)
    NQ = q // qt         # q tiles (4)
    SC = s // P          # s chunks (16)
    HC = hs // P         # hsize-slice chunks (2)
    DVE_SC = SC - POOL_SC
    assert hs % P == 0 and q % qt == 0 and qt == 512 and d == 1024

    nc = bacc.Bacc("TRN2", target_bir_lowering=False, debug=False,
                   num_devices=N_CORES)

    qT = nc.dram_tensor("qT", [d, q], bf16, kind="ExternalInput").ap()
    kT = nc.dram_tensor("kT", [d, s], bf16, kind="ExternalInput").ap()
    wqT = nc.dram_tensor("wqT", [d, hs], bf16, kind="ExternalInput").ap()
    wkT = nc.dram_tensor("wkT", [d, hs], bf16, kind="ExternalInput").ap()
    wvT = nc.dram_tensor("wvT", [d, hs], bf16, kind="ExternalInput").ap()
    woT = nc.dram_tensor("woT", [hs, d], bf16, kind="ExternalInput").ap()
    maskT = nc.dram_tensor("maskT", [s, q], bf16, kind="ExternalInput").ap()
    nbias = nc.dram_tensor("nbias", [1, 1], fp32, kind="ExternalInput").ap()
    out = nc.dram_tensor("out", [q, d], bf16, kind="ExternalOutput").ap()

    qT_t = qT.rearrange("(c p) q -> c p q", p=P)        # [8, 128, q]
    kT_t = kT.rearrange("(c p) s -> c p s", p=P)
    wqT_t = wqT.rearrange("(c p) h -> c p h", p=P)
    wkT_t = wkT.rearrange("(c p) h -> c p h", p=P)
    wvT_t = wvT.rearrange("(c p) h -> c p h", p=P)
    woT_t = woT.rearrange("(c p) d -> c p d", p=P)      # [2, 128, d]
    maskT_t = maskT.rearrange("(c p) q -> p c q", p=P)  # [128, SC, q]
    out_t = out.rearrange("(c p) d -> c p d", p=P)      # [q/P, 128, d]

    scale = 1.0 / np.sqrt(np.float32(adim))

    with tile.TileContext(nc) as tc, ExitStack() as ctx:
        const = ctx.enter_context(tc.tile_pool(name="const", bufs=1))
        wpool = ctx.enter_context(tc.tile_pool(name="w", bufs=1))
        xpool = ctx.enter_context(tc.tile_pool(name="x", bufs=9))
        actp = ctx.enter_context(tc.tile_pool(name="act", bufs=1))
        maskp = ctx.enter_context(tc.tile_pool(name="mask", bufs=2))
        upool = ctx.enter_context(tc.tile_pool(name="u", bufs=2))
        tpool = ctx.enter_context(tc.tile_pool(name="t", bufs=2))
        recp = ctx.enter_context(tc.tile_pool(name="rec", bufs=3))
        rbp = ctx.enter_context(tc.tile_pool(name="rb", bufs=2))
        outp = ctx.enter_context(tc.tile_pool(name="out", bufs=2))
        psB = ctx.enter_context(tc.tile_pool(name="psB", bufs=3, space="PSUM"))
        psS = ctx.enter_context(tc.tile_pool(name="psS", bufs=2, space="PSUM"))

        # ---- constants ----
        ones1 = const.tile([1, P], fp32)
        nc.any.memset(ones1[:], 1.0)
        nb1 = const.tile([1, 1], fp32)
        nc.sync.dma_start(nb1[:], nbias[:])
        ps_nb = psS.tile([P, 512], fp32, tag="ps", name="psnb")
        nc.tensor.matmul(ps_nb[:, 0:1], ones1[:], nb1[:], start=True, stop=True)
        nb128 = const.tile([P, 1], fp32)
        nc.scalar.copy(nb128[:], ps_nb[:, 0:1])

        # ---- resident weights on the ACT DMA queue so the sync queue is
        # free for the x chunks from t=0 ----
        wq_sb = wpool.tile([P, DC, hs], bf16)
        wk_sb = wpool.tile([P, DC, hs], bf16)
        wv_sb = wpool.tile([P, DC, hs], bf16)
        wo_sb = wpool.tile([P, HC, d], bf16)
        for c in range(DC):
            nc.scalar.dma_start(wq_sb[:, c], wqT_t[c])
        for c in range(DC):
            nc.scalar.dma_start(wk_sb[:, c], wkT_t[c])
            nc.scalar.dma_start(wv_sb[:, c], wvT_t[c])
        for c in range(HC):
            nc.gpsimd.dma_start(wo_sb[:, c], woT_t[c])

        # ---- resident activations ----
        rqT_sb = actp.tile([P, HC, q], bf16)    # (hs, q)
        rkT_sb = actp.tile([P, HC, s], bf16)    # (hs, s)
        rv_sb = actp.tile([P, SC, hpg * (adim + 1)], bf16)  # (s, hs + ones)
        oT_sb = actp.tile([P, HC, q], bf16)     # (hs, q)
        nc.gpsimd.memset(rv_sb[:], 1.0)         # ones cols survive at 64::65

        # first q tile's mask goes on the ACT queue, immediately
        mblk0 = maskp.tile([P, SC, qt], bf16, tag="m", name="mb0")
        nc.scalar.dma_start(mblk0[:], maskT_t[:, :, 0:qt])

        # ---- stage A inputs on the sync queue; xk reuses xq buffers so its
        # DMAs stream in as Qm1 frees them chunk by chunk ----
        xq = []
        for c in range(DC):
            xt = xpool.tile([P, q], bf16, tag="x", name=f"xq{c}")
            nc.sync.dma_start(xt[:], qT_t[c])
            xq.append(xt)
        xk = []
        for c in range(DC):
            xt = xpool.tile([P, s], bf16, tag="x", name=f"xk{c}")
            nc.sync.dma_start(xt[:], kT_t[c])
            xk.append(xt)

        def proj_block(m, w_sb, x_tiles, out_sb, on_act, sc_=1.0):
            """out_sb[:, m, :] = (W_m @ X); matmul N capped at one PSUM bank."""
            for t2i in range(q // 1024):
                ps = psB.tile([P, 1024], fp32, tag="pb", name="pproj")
                for c in range(DC):
                    for j in (0, 1):
                        lo = t2i * 1024 + j * 512
                        nc.tensor.matmul(
                            ps[:, j * 512:(j + 1) * 512],
                            w_sb[:, c, m * P:(m + 1) * P],
                            x_tiles[c][:, lo:lo + 512],
                            start=(c == 0), stop=(c == DC - 1))
                sl = out_sb[:, m, t2i * 1024:(t2i + 1) * 1024]
                if on_act:
                    nc.scalar.activation(sl, ps[:], AF.Copy, scale=float(sc_))
                else:
                    nc.vector.tensor_copy(sl, ps[:])

        def rv_block():
            for sc in range(SC):
                ps = psS.tile([P, 512], fp32, tag="ps", name="prv")
                for c in range(DC):
                    nc.tensor.matmul(
                        ps[:, :hs], xk[c][:, sc * P:(sc + 1) * P],
                        wv_sb[:, c], start=(c == 0), stop=(c == DC - 1))
                nc.vector.tensor_copy(
                    rv_sb[:, sc].rearrange("p (h c) -> p h c", c=adim + 1)[:, :, 0:adim],
                    ps[:, :hs].rearrange("p (h c) -> p h c", c=adim))

        # ---- middle-phase blocks ----
        # AV chunks of the previous head are interleaved between score pairs,
        # ordered by t-chunk readiness (j2's mask runs on GpSimd and lands
        # last), so the PE always has ready work between relu-gated pairs.
        AV_SCHED = {0: (0, 4), 1: (4, 8), 2: (12, 16), 4: (8, 12)}

        def scores_block(h, qlo, u, avw=None):
            hp, hc = (h % 2) * adim, h // 2
            po = None
            n_av = 0
            if avw is not None:
                hprev, qloprev, tprev = avw
                po = psS.tile([P, 512], fp32, tag="ps", name="pav")
            for k in range(8):          # sc pairs
                if po is not None and k in AV_SCHED:
                    lo, hi = AV_SCHED[k]
                    for sc in range(lo, hi):
                        nc.tensor.matmul(
                            po[0:adim + 1, :],
                            rv_sb[:, sc, hprev * (adim + 1):(hprev + 1) * (adim + 1)],
                            tprev[:, sc], start=(n_av == 0), stop=(n_av == SC - 1))
                        n_av += 1
                ps = psB.tile([P, 1024], fp32, tag="pb", name="pscore")
                for j in (0, 1):
                    sc = 2 * k + j
                    nc.tensor.matmul(
                        ps[:, j * 512:(j + 1) * 512],
                        rkT_sb[hp:hp + adim, hc, sc * P:(sc + 1) * P],
                        rqT_sb[hp:hp + adim, hc, qlo:qlo + qt],
                        start=True, stop=True)
                up = u[:, 2 * k:2 * k + 2].rearrange("p a b -> p (a b)")
                nc.scalar.activation(up, ps[:], AF.Relu, bias=nb128[:])
            return po

        def ew_block(h, u, t, mblk):
            """u *= u in place (square), then t = u * mask, in 4-sc chunks.
            GpSimd takes the last square chunk and the j=2 mask chunk; they
            sit off the critical chain (AV consumes those s-chunks last)."""
            for j in range(4):
                u4 = u[:, 4 * j:4 * j + 4].rearrange("p a b -> p (a b)")
                eng = nc.gpsimd if j == 3 else nc.vector
                eng.tensor_mul(u4, u4, u4)
            for j in range(4):
                u4 = u[:, 4 * j:4 * j + 4].rearrange("p a b -> p (a b)")
                m4 = mblk[:, 4 * j:4 * j + 4].rearrange("p a b -> p (a b)")
                t4 = t[:, 4 * j:4 * j + 4].rearrange("p a b -> p (a b)")
                eng = nc.gpsimd if j == 2 else nc.vector
                eng.tensor_mul(t4, u4, m4)

        def av_mm(h, qlo, t):
            po = psS.tile([P, 512], fp32, tag="ps", name="pav")
            for sc in range(SC):
                nc.tensor.matmul(
                    po[0:adim + 1, :],
                    rv_sb[:, sc, h * (adim + 1):(h + 1) * (adim + 1)],
                    t[:, sc], start=(sc == 0), stop=(sc == SC - 1))
            return (h, qlo, po)

        def av_rec(h, qlo, po):
            """rec = 1/den on the ACT Reciprocal unit, via a raw instruction
            (the bass wrapper refuses it; its accuracy is amply within our
            2e-2 budget, and it shares an act table with Relu/Copy). The DVE
            reciprocal costs 3.4us per [1,512] row - far too slow."""
            rec = recp.tile([1, 512], fp32, tag="rec", name="rect")
            se = nc.scalar
            ins_l = [se.lower_ap(po[adim:adim + 1, :])]
            for arg in (0.0, 1.0, 0.0):     # bias, scale, alpha
                ins_l.append(mybir.ImmediateValue(dtype=fp32, value=arg))
            se.add_instruction(mybir.InstActivation(
                name=se.bass.get_next_instruction_name(),
                func=AF.Reciprocal, ins=ins_l, outs=[se.lower_ap(rec[:])]))
            return (h, qlo, po, rec)

        def av_fin(h, qlo, po, rec):
            hp, hc = (h % 2) * adim, h // 2
            rb_sb = rbp.tile([adim, 512], fp32, tag="rb", name="rbt")
            nc.gpsimd.partition_broadcast(rb_sb[:], rec[:])
            nc.vector.tensor_mul(oT_sb[hp:hp + adim, hc, qlo:qlo + qt],
                                 po[0:adim, :], rb_sb[:])

        def outproj_qc(iq, qc):
            qlo = iq * qt
            ps = psB.tile([P, 1024], fp32, tag="pb", name="pout")
            for c in range(HC):
                for j in (0, 1):
                    nc.tensor.matmul(
                        ps[:, j * 512:(j + 1) * 512],
                        oT_sb[:, c, qlo + qc * P:qlo + (qc + 1) * P],
                        wo_sb[:, c, j * 512:(j + 1) * 512],
                        start=(c == 0), stop=(c == HC - 1))
            ob = outp.tile([P, 1024], bf16, tag="ob", name="obt")
            if qc % 2 == 0:
                nc.scalar.copy(ob[:], ps[:])
            else:
                nc.vector.tensor_copy(ob[:], ps[:])
            nc.sync.dma_start(out_t[iq * (qt // P) + qc], ob[:])

        # ---- stage A ----
        proj_block(0, wq_sb, xq, rqT_sb, on_act=True, sc_=scale)
        proj_block(1, wq_sb, xq, rqT_sb, on_act=True, sc_=scale)
        proj_block(0, wk_sb, xk, rkT_sb, on_act=False)

        # ---- middle: units pipelined (AV mm of h-1 first: a ready PE burst;
        # av_fin of h-2; one out-projection chunk; scores of h; rec of h-1;
        # elementwise of h) ----
        prev = None           # (h, qlo, t) awaiting av_mm
        fin = None            # av_rec result awaiting av_fin
        fin_h3, fin_iq = False, 0
        pending_out = []      # (iq, qc) out-projection chunks ready to emit
        for iq in range(NQ):
            qlo = iq * qt
            if iq == 0:
                mblk = mblk0
            else:
                mblk = maskp.tile([P, SC, qt], bf16, tag="m", name=f"mb{iq}")
                nc.scalar.dma_start(mblk[:], maskT_t[:, :, qlo:qlo + qt])
            for h in range(hpg):
                u = upool.tile([P, SC, qt], bf16, tag="u", name="ut")
                t = tpool.tile([P, SC, qt], bf16, tag="t", name="tt")
                if fin is not None:
                    av_fin(*fin)
                    fin = None
                    if fin_h3:
                        pending_out.extend((fin_iq, qc) for qc in range(4))
                if iq == 0 and h == 1:
                    rv_block()      # before the AV matmuls, which read rv_sb
                po = scores_block(h, qlo, u, avw=prev)
                if pending_out and h == 2:
                    while pending_out:
                        outproj_qc(*pending_out.pop(0))
                if iq == 0 and h == 0:
                    proj_block(1, wk_sb, xk, rkT_sb, on_act=False)
                if po is not None:
                    ph, pq, _ = prev
                    fin = av_rec(ph, pq, po)
                    fin_h3, fin_iq = (ph == hpg - 1), pq // qt
                ew_block(h, u, t, mblk)
                prev = (h, qlo, t)
        # tail: drain the pipeline
        av_fin(*fin)
        mids = av_mm(*prev)
        fin = av_rec(*mids)
        av_fin(*fin)
        pending_out.extend((NQ - 1, qc) for qc in range(4))
        for w in pending_out:
            outproj_qc(*w)

    nc.compile()
    return nc


def _shard_inputs(iQ, iK, mask, Wq, Wkv, Wo, nbias):
    in_maps = []
    maskT_by_b = [np.ascontiguousarray((~mask[b]).T).astype(BF16)
                  for b in range(B)]
    qT_by_b = [np.ascontiguousarray(iQ[b].T).astype(BF16) for b in range(B)]
    kT_by_b = [np.ascontiguousarray(iK[b].T).astype(BF16) for b in range(B)]
    nb = np.asarray(nbias, np.float32).reshape(1, 1)
    for ci in range(N_CORES):
        b, g = ci // GROUPS, ci % GROUPS
        hsl = slice(g * HS, (g + 1) * HS)
        in_maps.append({
            "qT": qT_by_b[b],
            "kT": kT_by_b[b],
            "wqT": np.ascontiguousarray(Wq[hsl].T).astype(BF16),
            "wkT": np.ascontiguousarray(Wkv[hsl].T).astype(BF16),
            "wvT": np.ascontiguousarray(Wkv[HSIZE + g * HS:HSIZE + (g + 1) * HS].T).astype(BF16),
            "woT": np.ascontiguousarray(Wo[:, hsl].T).astype(BF16),
            "maskT": maskT_by_b[b],
            "nbias": nb,
        })
    return in_maps


def kernel(iQ, iK, mask, Wq, Wkv, Wo, nbias):
    global _COMPILED
    from concourse.bass_utils import run_bass_kernel_spmd

    if _COMPILED is None:
        _COMPILED = _build()
    in_maps = _shard_inputs(np.asarray(iQ, np.float32), np.asarray(iK, np.float32),
                            np.asarray(mask), np.asarray(Wq, np.float32),
                            np.asarray(Wkv, np.float32), np.asarray(Wo, np.float32),
                            np.asarray(nbias, np.float32))
    res = run_bass_kernel_spmd(_COMPILED, in_maps, list(range(N_CORES))).results
    out = np.zeros((B, Q, D), np.float32)
    for ci in range(N_CORES):
        out[ci // GROUPS] += np.asarray(res[ci]["out"], np.float32)
    return out


# revision 37
# speedup vs baseline: 1.0627x; 1.0627x over previous
"""Sparse cross-attention (squared-ReLU normalizer) on 8 TRN2 NeuronCores.

Sharding: 8 cores = batch(2) x head-group(4). Each core owns one batch and
4 of 16 heads (a 256-wide slice of hsize): Wq/Wkv column-parallel,
Wo row-parallel (partial outputs summed on host), mask replicated per
batch shard.

Per-core kernel, engine-balanced (bf16 matmuls, fp32 PSUM):
  stage A: rqT (hs,q), rkT (hs,s) via weight-stationary projections into
    [128,1024] PSUM tiles; rv (s, hs+ones) with kT chunks as weights.
    rq evicted on ACT (scale 1/sqrt(a) folded), rk/rv evicted on DVE.
    Input DMA issues are spread across engine queues (weights on DVE,
    mask on ACT, x on sync) so transfers overlap from t=0.
  middle, per unit = (q-tile 512, head): AV matmuls of the previous head
    run first (their t is ready - a contiguous PE burst), then one out-
    projection chunk, then 8 score-matmul pairs -> [128,1024] PSUM, relu
    +nbias on ACT -> u; square IN PLACE (u*=u) and mask mul (t=u*maskT)
    on DVE with the tail s-chunks on GpSimd. AV uses rv chunks as
    weights giving oT (64,q) + denominator row; 1/den via ACT exp(-ln);
    GpSimd partition-broadcast spreads rec; DVE normalizes+evicts oT.
  out projection: oT^T @ woT in PSUM, evicted bf16 (ACT/DVE alternating),
    DMA'd to DRAM; host sums the 4 row-parallel partials in fp32.
"""

import numpy as np
import ml_dtypes

BF16 = ml_dtypes.bfloat16

B, Q, S, D = 2, 2048, 2048, 1024
NUM_HEAD, ADIM = 16, 64
HSIZE = NUM_HEAD * ADIM
N_CORES = 8
GROUPS = 4                  # head groups (tensor-parallel dim)
HPG = NUM_HEAD // GROUPS    # 4 heads per core
HS = HPG * ADIM             # 256: per-core hsize slice
P = 128

POOL_SC = 3                 # trailing s-chunks per mul pass on GpSimd

_COMPILED = None


def _build(q=Q, s=S, d=D, hpg=HPG, adim=ADIM, qt=512):
    """Build + compile the per-core Bass program. Returns the Bacc."""
    from contextlib import ExitStack
    import concourse.bass as bass
    import concourse.mybir as mybir
    import concourse.tile as tile
    from concourse import bacc

    fp32 = mybir.dt.float32
    bf16 = mybir.dt.bfloat16
    AF = mybir.ActivationFunctionType

    hs = hpg * adim
    DC = d // P          # contraction chunks for projections (8)
    NQ = q // qt         # q tiles (4)
    SC = s // P          # s chunks (16)
    HC = hs // P         # hsize-slice chunks (2)
    DVE_SC = SC - POOL_SC
    assert hs % P == 0 and q % qt == 0 and qt == 512 and d == 1024

    nc = bacc.Bacc("TRN2", target_bir_lowering=False, debug=False,
                   num_devices=N_CORES)

    qT = nc.dram_tensor("qT", [d, q], bf16, kind="ExternalInput").ap()
    kT = nc.dram_tensor("kT", [d, s], bf16, kind="ExternalInput").ap()
    wqT = nc.dram_tensor("wqT", [d, hs], bf16, kind="ExternalInput").ap()
    wkT = nc.dram_tensor("wkT", [d, hs], bf16, kind="ExternalInput").ap()
    wvT = nc.dram_tensor("wvT", [d, hs], bf16, kind="ExternalInput").ap()
    woT = nc.dram_tensor("woT", [hs, d], bf16, kind="ExternalInput").ap()
    maskT = nc.dram_tensor("maskT", [s, q], bf16, kind="ExternalInput").ap()
    nbias = nc.dram_tensor("nbias", [1, 1], fp32, kind="ExternalInput").ap()
    out = nc.dram_tensor("out", [q, d], bf16, kind="ExternalOutput").ap()

    qT_t = qT.rearrange("(c p) q -> c p q", p=P)        # [8, 128, q]
    kT_t = kT.rearrange("(c p) s -> c p s", p=P)
    wqT_t = wqT.rearrange("(c p) h -> c p h", p=P)
    wkT_t = wkT.rearrange("(c p) h -> c p h", p=P)
    wvT_t = wvT.rearrange("(c p) h -> c p h", p=P)
    woT_t = woT.rearrange("(c p) d -> c p d", p=P)      # [2, 128, d]
    maskT_t = maskT.rearrange("(c p) q -> p c q", p=P)  # [128, SC, q]
    out_t = out.rearrange("(c p) d -> c p d", p=P)      # [q/P, 128, d]

    scale = 1.0 / np.sqrt(np.float32(adim))

    with tile.TileContext(nc) as tc, ExitStack() as ctx:
        const = ctx.enter_context(tc.tile_pool(name="const", bufs=1))
        wpool = ctx.enter_context(tc.tile_pool(name="w", bufs=1))
        xpool = ctx.enter_context(tc.tile_pool(name="x", bufs=9))
        actp = ctx.enter_context(tc.tile_pool(name="act", bufs=1))
        maskp = ctx.enter_context(tc.tile_pool(name="mask", bufs=2))
        upool = ctx.enter_context(tc.tile_pool(name="u", bufs=3))
        recp = ctx.enter_context(tc.tile_pool(name="rec", bufs=3))
        rbp = ctx.enter_context(tc.tile_pool(name="rb", bufs=2))
        outp = ctx.enter_context(tc.tile_pool(name="out", bufs=2))
        psB = ctx.enter_context(tc.tile_pool(name="psB", bufs=3, space="PSUM"))
        psS = ctx.enter_context(tc.tile_pool(name="psS", bufs=2, space="PSUM"))

        # ---- constants ----
        ones1 = const.tile([1, P], fp32)
        nc.any.memset(ones1[:], 1.0)
        nb1 = const.tile([1, 1], fp32)
        nc.sync.dma_start(nb1[:], nbias[:])
        ps_nb = psS.tile([P, 512], fp32, tag="ps", name="psnb")
        nc.tensor.matmul(ps_nb[:, 0:1], ones1[:], nb1[:], start=True, stop=True)
        nb128 = const.tile([P, 1], fp32)
        nc.scalar.copy(nb128[:], ps_nb[:, 0:1])

        # ---- resident weights on the ACT DMA queue so the sync queue is
        # free for the x chunks from t=0 ----
        wq_sb = wpool.tile([P, DC, hs], bf16)
        wk_sb = wpool.tile([P, DC, hs], bf16)
        wv_sb = wpool.tile([P, DC, hs], bf16)
        wo_sb = wpool.tile([P, HC, d], bf16)
        for c in range(DC):
            nc.sync.dma_start(wq_sb[:, c], wqT_t[c])

        # ---- resident activations ----
        rqT_sb = actp.tile([P, HC, q], bf16)    # (hs, q)
        rkT_sb = actp.tile([P, HC, s], bf16)    # (hs, s)
        rv_sb = actp.tile([P, SC, hpg * (adim + 1)], bf16)  # (s, hs + ones)
        oT_sb = actp.tile([P, HC, q], bf16)     # (hs, q)
        nc.gpsimd.memset(rv_sb[:], 1.0)         # ones cols survive at 64::65

        # ---- stage A inputs; mask for the first q tile is hoisted before
        # the K-side loads; xk reuses xq buffers so its DMAs stream in as
        # Qm1 frees them chunk by chunk ----
        xq = []
        for c in range(DC):
            xt = xpool.tile([P, q], bf16, tag="x", name=f"xq{c}")
            nc.sync.dma_start(xt[:], qT_t[c])
            xq.append(xt)
        mblk0 = maskp.tile([P, SC, qt], bf16, tag="m", name="mb0")
        nc.sync.dma_start(mblk0[:], maskT_t[:, :, 0:qt])
        for c in range(DC):
            nc.sync.dma_start(wk_sb[:, c], wkT_t[c])
            nc.sync.dma_start(wv_sb[:, c], wvT_t[c])
        xk = []
        for c in range(DC):
            xt = xpool.tile([P, s], bf16, tag="x", name=f"xk{c}")
            nc.sync.dma_start(xt[:], kT_t[c])
            xk.append(xt)
        for c in range(HC):
            nc.sync.dma_start(wo_sb[:, c], woT_t[c])

        def proj_block(m, w_sb, x_tiles, out_sb, on_act, sc_=1.0):
            """out_sb[:, m, :] = (W_m @ X); matmul N capped at one PSUM bank."""
            for t2i in range(q // 1024):
                ps = psB.tile([P, 1024], fp32, tag="pb", name="pproj")
                for c in range(DC):
                    for j in (0, 1):
                        lo = t2i * 1024 + j * 512
                        nc.tensor.matmul(
                            ps[:, j * 512:(j + 1) * 512],
                            w_sb[:, c, m * P:(m + 1) * P],
                            x_tiles[c][:, lo:lo + 512],
                            start=(c == 0), stop=(c == DC - 1))
                sl = out_sb[:, m, t2i * 1024:(t2i + 1) * 1024]
                if on_act:
                    nc.scalar.activation(sl, ps[:], AF.Copy, scale=float(sc_))
                else:
                    nc.vector.tensor_copy(sl, ps[:])

        def rv_block():
            for sc in range(SC):
                ps = psS.tile([P, 512], fp32, tag="ps", name="prv")
                for c in range(DC):
                    nc.tensor.matmul(
                        ps[:, :hs], xk[c][:, sc * P:(sc + 1) * P],
                        wv_sb[:, c], start=(c == 0), stop=(c == DC - 1))
                nc.vector.tensor_copy(
                    rv_sb[:, sc].rearrange("p (h c) -> p h c", c=adim + 1)[:, :, 0:adim],
                    ps[:, :hs].rearrange("p (h c) -> p h c", c=adim))

        # ---- middle-phase blocks ----
        # AV chunks from TWO units back are interleaved between score pairs:
        # their t is fully computed, so they never stall the in-order PE and
        # fill the relu-gated gaps between pairs (keeps the clock ramped).
        AV_SCHED = {0: (0, 4), 1: (4, 8), 2: (8, 12), 3: (12, 16)}

        def scores_block(h, qlo, u, avw=None):
            hp, hc = (h % 2) * adim, h // 2
            po = None
            n_av = 0
            if avw is not None:
                hprev, qloprev, tprev = avw
                po = psS.tile([P, 512], fp32, tag="ps", name="pav")
            for k in range(8):          # sc pairs
                if po is not None and k in AV_SCHED:
                    lo, hi = AV_SCHED[k]
                    for sc in range(lo, hi):
                        nc.tensor.matmul(
                            po[0:adim + 1, :],
                            rv_sb[:, sc, hprev * (adim + 1):(hprev + 1) * (adim + 1)],
                            tprev[:, sc], start=(n_av == 0), stop=(n_av == SC - 1))
                        n_av += 1
                ps = psB.tile([P, 1024], fp32, tag="pb", name="pscore")
                for j in (0, 1):
                    sc = 2 * k + j
                    nc.tensor.matmul(
                        ps[:, j * 512:(j + 1) * 512],
                        rkT_sb[hp:hp + adim, hc, sc * P:(sc + 1) * P],
                        rqT_sb[hp:hp + adim, hc, qlo:qlo + qt],
                        start=True, stop=True)
                up = u[:, 2 * k:2 * k + 2].rearrange("p a b -> p (a b)")
                nc.scalar.activation(up, ps[:], AF.Relu, bias=nb128[:])
            return po

        def ew_block(h, u, mblk):
            """Fully in place: u *= u (square), then u *= mask, in 4-sc
            chunks; u then IS t and feeds the AV matmuls two units later.
            GpSimd takes one chunk of each pass."""
            for j in range(4):
                u4 = u[:, 4 * j:4 * j + 4].rearrange("p a b -> p (a b)")
                eng = nc.gpsimd if j == 3 else nc.vector
                eng.tensor_mul(u4, u4, u4)
            for j in range(4):
                u4 = u[:, 4 * j:4 * j + 4].rearrange("p a b -> p (a b)")
                m4 = mblk[:, 4 * j:4 * j + 4].rearrange("p a b -> p (a b)")
                eng = nc.gpsimd if j == 2 else nc.vector
                eng.tensor_mul(u4, u4, m4)

        def av_mm(h, qlo, t):
            po = psS.tile([P, 512], fp32, tag="ps", name="pav")
            for sc in range(SC):
                nc.tensor.matmul(
                    po[0:adim + 1, :],
                    rv_sb[:, sc, h * (adim + 1):(h + 1) * (adim + 1)],
                    t[:, sc], start=(sc == 0), stop=(sc == SC - 1))
            return (h, qlo, po)

        def av_rec(h, qlo, po):
            """rec = 1/den on the ACT Reciprocal unit, via a raw instruction
            (the bass wrapper refuses it; its accuracy is amply within our
            2e-2 budget, and it shares an act table with Relu/Copy). The DVE
            reciprocal costs 3.4us per [1,512] row - far too slow."""
            rec = recp.tile([1, 512], fp32, tag="rec", name="rect")
            se = nc.scalar
            ins_l = [se.lower_ap(po[adim:adim + 1, :])]
            for arg in (0.0, 1.0, 0.0):     # bias, scale, alpha
                ins_l.append(mybir.ImmediateValue(dtype=fp32, value=arg))
            se.add_instruction(mybir.InstActivation(
                name=se.bass.get_next_instruction_name(),
                func=AF.Reciprocal, ins=ins_l, outs=[se.lower_ap(rec[:])]))
            return (h, qlo, po, rec)

        def av_fin(h, qlo, po, rec):
            hp, hc = (h % 2) * adim, h // 2
            rb_sb = rbp.tile([adim, 512], fp32, tag="rb", name="rbt")
            nc.gpsimd.partition_broadcast(rb_sb[:], rec[:])
            nc.vector.tensor_mul(oT_sb[hp:hp + adim, hc, qlo:qlo + qt],
                                 po[0:adim, :], rb_sb[:])

        def outproj_qc(iq, qc):
            qlo = iq * qt
            ps = psB.tile([P, 1024], fp32, tag="pb", name="pout")
            for c in range(HC):
                for j in (0, 1):
                    nc.tensor.matmul(
                        ps[:, j * 512:(j + 1) * 512],
                        oT_sb[:, c, qlo + qc * P:qlo + (qc + 1) * P],
                        wo_sb[:, c, j * 512:(j + 1) * 512],
                        start=(c == 0), stop=(c == HC - 1))
            ob = outp.tile([P, 1024], bf16, tag="ob", name="obt")
            nc.vector.tensor_copy(ob[:], ps[:])
            nc.sync.dma_start(out_t[iq * (qt // P) + qc], ob[:])

        # ---- stage A ----
        proj_block(0, wq_sb, xq, rqT_sb, on_act=True, sc_=scale)
        proj_block(1, wq_sb, xq, rqT_sb, on_act=True, sc_=scale)
        proj_block(0, wk_sb, xk, rkT_sb, on_act=False)

        # ---- middle: units two-deep pipelined. Unit (iq,h): av_fin of the
        # unit three back; AV matmuls of the unit two back interleaved with
        # this unit's score pairs; reciprocal of the popped unit; in-place
        # elementwise of this unit. Out-projection of q tile iq-1 flushed at
        # h==3 (after its last av_fin). ----
        av_q = []             # (h, qlo, u) units awaiting AV matmuls
        fin = None            # av_rec result awaiting av_fin
        fin_h3, fin_iq = False, 0
        pending_out = []      # (iq, qc) out-projection chunks ready to emit
        for iq in range(NQ):
            qlo = iq * qt
            if iq == 0:
                mblk = mblk0
            else:
                mblk = maskp.tile([P, SC, qt], bf16, tag="m", name=f"mb{iq}")
                nc.sync.dma_start(mblk[:], maskT_t[:, :, qlo:qlo + qt])
            for h in range(hpg):
                u = upool.tile([P, SC, qt], bf16, tag="u", name="ut")
                if fin is not None:
                    av_fin(*fin)
                    fin = None
                    if fin_h3:
                        pending_out.extend((fin_iq, qc) for qc in range(4))
                if iq == 0 and h == 1:
                    rv_block()      # before the AV matmuls, which read rv_sb
                avw = av_q.pop(0) if len(av_q) >= 2 else None
                po = scores_block(h, qlo, u, avw=avw)
                if pending_out and h == 3:
                    while pending_out:
                        outproj_qc(*pending_out.pop(0))
                if iq == 0 and h == 0:
                    proj_block(1, wk_sb, xk, rkT_sb, on_act=False)
                if po is not None:
                    fin = av_rec(avw[0], avw[1], po)
                    fin_h3, fin_iq = (avw[0] == hpg - 1), avw[1] // qt
                ew_block(h, u, mblk)
                av_q.append((h, qlo, u))
        # tail: drain the two-deep pipeline
        if fin is not None:
            av_fin(*fin)
        for (hh, qq, uu) in av_q:
            _, _, po = av_mm(hh, qq, uu)
            f = av_rec(hh, qq, po)
            av_fin(*f)
        pending_out.extend((NQ - 1, qc) for qc in range(4))
        for w in pending_out:
            outproj_qc(*w)

    nc.compile()
    return nc


def _shard_inputs(iQ, iK, mask, Wq, Wkv, Wo, nbias):
    in_maps = []
    maskT_by_b = [np.ascontiguousarray((~mask[b]).T).astype(BF16)
                  for b in range(B)]
    qT_by_b = [np.ascontiguousarray(iQ[b].T).astype(BF16) for b in range(B)]
    kT_by_b = [np.ascontiguousarray(iK[b].T).astype(BF16) for b in range(B)]
    nb = np.asarray(nbias, np.float32).reshape(1, 1)
    for ci in range(N_CORES):
        b, g = ci // GROUPS, ci % GROUPS
        hsl = slice(g * HS, (g + 1) * HS)
        in_maps.append({
            "qT": qT_by_b[b],
            "kT": kT_by_b[b],
            "wqT": np.ascontiguousarray(Wq[hsl].T).astype(BF16),
            "wkT": np.ascontiguousarray(Wkv[hsl].T).astype(BF16),
            "wvT": np.ascontiguousarray(Wkv[HSIZE + g * HS:HSIZE + (g + 1) * HS].T).astype(BF16),
            "woT": np.ascontiguousarray(Wo[:, hsl].T).astype(BF16),
            "maskT": maskT_by_b[b],
            "nbias": nb,
        })
    return in_maps


def kernel(iQ, iK, mask, Wq, Wkv, Wo, nbias):
    global _COMPILED
    from concourse.bass_utils import run_bass_kernel_spmd

    if _COMPILED is None:
        _COMPILED = _build()
    in_maps = _shard_inputs(np.asarray(iQ, np.float32), np.asarray(iK, np.float32),
                            np.asarray(mask), np.asarray(Wq, np.float32),
                            np.asarray(Wkv, np.float32), np.asarray(Wo, np.float32),
                            np.asarray(nbias, np.float32))
    res = run_bass_kernel_spmd(_COMPILED, in_maps, list(range(N_CORES))).results
    out = np.zeros((B, Q, D), np.float32)
    for ci in range(N_CORES):
        out[ci // GROUPS] += np.asarray(res[ci]["out"], np.float32)
    return out


# revision 40
# speedup vs baseline: 1.3475x; 1.2680x over previous
"""Sparse cross-attention (squared-ReLU normalizer) on 8 TRN2 NeuronCores.

Sharding: 8 cores = batch(2) x head-group(4). Each core owns one batch and
4 of 16 heads (a 256-wide slice of hsize): Wq/Wkv column-parallel,
Wo row-parallel (partial outputs summed on host), mask replicated per
batch shard.

Per-core kernel, engine-balanced (bf16 matmuls, fp32 PSUM):
  stage A: rqT (hs,q), rkT (hs,s) via weight-stationary projections into
    [128,1024] PSUM tiles; rv (s, hs+ones) with kT chunks as weights.
    rq evicted on ACT (scale 1/sqrt(a) folded), rk/rv evicted on DVE.
    Input DMA issues are spread across engine queues (weights on DVE,
    mask on ACT, x on sync) so transfers overlap from t=0.
  middle, per unit = (q-tile 512, head): AV matmuls of the previous head
    run first (their t is ready - a contiguous PE burst), then one out-
    projection chunk, then 8 score-matmul pairs -> [128,1024] PSUM, relu
    +nbias on ACT -> u; square IN PLACE (u*=u) and mask mul (t=u*maskT)
    on DVE with the tail s-chunks on GpSimd. AV uses rv chunks as
    weights giving oT (64,q) + denominator row; 1/den via ACT exp(-ln);
    GpSimd partition-broadcast spreads rec; DVE normalizes+evicts oT.
  out projection: oT^T @ woT in PSUM, evicted bf16 (ACT/DVE alternating),
    DMA'd to DRAM; host sums the 4 row-parallel partials in fp32.
"""

import numpy as np
import ml_dtypes

BF16 = ml_dtypes.bfloat16

B, Q, S, D = 2, 2048, 2048, 1024
NUM_HEAD, ADIM = 16, 64
HSIZE = NUM_HEAD * ADIM
N_CORES = 8
GROUPS = 4                  # head groups (tensor-parallel dim)
HPG = NUM_HEAD // GROUPS    # 4 heads per core
HS = HPG * ADIM             # 256: per-core hsize slice
P = 128

POOL_SC = 3                 # trailing s-chunks per mul pass on GpSimd

_COMPILED = None


def _build(q=Q, s=S, d=D, hpg=HPG, adim=ADIM, qt=512):
    """Build + compile the per-core Bass program. Returns the Bacc."""
    from contextlib import ExitStack
    import concourse.bass as bass
    import concourse.mybir as mybir
    import concourse.tile as tile
    from concourse import bacc

    fp32 = mybir.dt.float32
    bf16 = mybir.dt.bfloat16
    AF = mybir.ActivationFunctionType

    hs = hpg * adim
    DC = d // P          # contraction chunks for projections (8)
    NQ = q // qt         # q tiles (4)
    SC = s // P          # s chunks (16)
    HC = hs // P         # hsize-slice chunks (2)
    DVE_SC = SC - POOL_SC
    assert hs % P == 0 and q % qt == 0 and qt == 512 and d == 1024

    nc = bacc.Bacc("TRN2", target_bir_lowering=False, debug=False,
                   num_devices=N_CORES)

    qT = nc.dram_tensor("qT", [d, q], bf16, kind="ExternalInput").ap()
    kT = nc.dram_tensor("kT", [d, s], bf16, kind="ExternalInput").ap()
    wqT = nc.dram_tensor("wqT", [d, hs], bf16, kind="ExternalInput").ap()
    wkT = nc.dram_tensor("wkT", [d, hs], bf16, kind="ExternalInput").ap()
    wvT = nc.dram_tensor("wvT", [d, hs], bf16, kind="ExternalInput").ap()
    woT = nc.dram_tensor("woT", [hs, d], bf16, kind="ExternalInput").ap()
    maskT = nc.dram_tensor("maskT", [s, q], bf16, kind="ExternalInput").ap()
    nbias = nc.dram_tensor("nbias", [1, 1], fp32, kind="ExternalInput").ap()
    out = nc.dram_tensor("out", [q, d], bf16, kind="ExternalOutput").ap()

    qT_t = qT.rearrange("(c p) q -> c p q", p=P)        # [8, 128, q]
    kT_t = kT.rearrange("(c p) s -> c p s", p=P)
    wqT_t = wqT.rearrange("(c p) h -> c p h", p=P)
    wkT_t = wkT.rearrange("(c p) h -> c p h", p=P)
    wvT_t = wvT.rearrange("(c p) h -> c p h", p=P)
    woT_t = woT.rearrange("(c p) d -> c p d", p=P)      # [2, 128, d]
    maskT_t = maskT.rearrange("(c p) q -> p c q", p=P)  # [128, SC, q]
    out_t = out.rearrange("(c p) d -> c p d", p=P)      # [q/P, 128, d]

    scale = 1.0 / np.sqrt(np.float32(adim))

    with tile.TileContext(nc) as tc, ExitStack() as ctx:
        const = ctx.enter_context(tc.tile_pool(name="const", bufs=1))
        wpool = ctx.enter_context(tc.tile_pool(name="w", bufs=1))
        xpool = ctx.enter_context(tc.tile_pool(name="x", bufs=9))
        actp = ctx.enter_context(tc.tile_pool(name="act", bufs=1))
        maskp = ctx.enter_context(tc.tile_pool(name="mask", bufs=2))
        upool = ctx.enter_context(tc.tile_pool(name="u", bufs=3))
        recp = ctx.enter_context(tc.tile_pool(name="rec", bufs=3))
        rbp = ctx.enter_context(tc.tile_pool(name="rb", bufs=2))
        outp = ctx.enter_context(tc.tile_pool(name="out", bufs=2))
        psB = ctx.enter_context(tc.tile_pool(name="psB", bufs=3, space="PSUM"))
        psS = ctx.enter_context(tc.tile_pool(name="psS", bufs=2, space="PSUM"))

        # ---- constants ----
        ones1 = const.tile([1, P], fp32)
        nc.any.memset(ones1[:], 1.0)
        nb1 = const.tile([1, 1], fp32)
        nc.sync.dma_start(nb1[:], nbias[:])
        ps_nb = psS.tile([P, 512], fp32, tag="ps", name="psnb")
        nc.tensor.matmul(ps_nb[:, 0:1], ones1[:], nb1[:], start=True, stop=True)
        nb128 = const.tile([P, 1], fp32)
        nc.scalar.copy(nb128[:], ps_nb[:, 0:1])

        # ---- resident weights on the ACT DMA queue so the sync queue is
        # free for the x chunks from t=0 ----
        wq_sb = wpool.tile([P, DC, hs], bf16)
        wk_sb = wpool.tile([P, DC, hs], bf16)
        wv_sb = wpool.tile([P, DC, hs], bf16)
        wo_sb = wpool.tile([P, HC, d], bf16)
        for c in range(DC):
            nc.sync.dma_start(wq_sb[:, c], wqT_t[c])

        # ---- resident activations ----
        rqT_sb = actp.tile([P, HC, q], bf16)    # (hs, q)
        rkT_sb = actp.tile([P, HC, s], bf16)    # (hs, s)
        rv_sb = actp.tile([P, SC, hpg * (adim + 1)], bf16)  # (s, hs + ones)
        oT_sb = actp.tile([P, HC, q], bf16)     # (hs, q)
        nc.gpsimd.memset(rv_sb[:], 1.0)         # ones cols survive at 64::65

        # ---- stage A inputs; mask for the first q tile is hoisted before
        # the K-side loads; xk reuses xq buffers so its DMAs stream in as
        # Qm1 frees them chunk by chunk ----
        xq = []
        for c in range(DC):
            xt = xpool.tile([P, q], bf16, tag="x", name=f"xq{c}")
            nc.sync.dma_start(xt[:], qT_t[c])
            xq.append(xt)
        mblk0 = maskp.tile([P, SC, qt], bf16, tag="m", name="mb0")
        nc.sync.dma_start(mblk0[:], maskT_t[:, :, 0:qt])
        for c in range(DC):
            nc.sync.dma_start(wk_sb[:, c], wkT_t[c])
            nc.sync.dma_start(wv_sb[:, c], wvT_t[c])
        xk = []
        for c in range(DC):
            xt = xpool.tile([P, s], bf16, tag="x", name=f"xk{c}")
            nc.sync.dma_start(xt[:], kT_t[c])
            xk.append(xt)
        for c in range(HC):
            nc.sync.dma_start(wo_sb[:, c], woT_t[c])

        def proj_block(m, w_sb, x_tiles, out_sb, on_act, sc_=1.0):
            """out_sb[:, m, :] = (W_m @ X); matmul N capped at one PSUM bank."""
            for t2i in range(q // 1024):
                ps = psB.tile([P, 1024], fp32, tag="pb", name="pproj")
                for c in range(DC):
                    for j in (0, 1):
                        lo = t2i * 1024 + j * 512
                        nc.tensor.matmul(
                            ps[:, j * 512:(j + 1) * 512],
                            w_sb[:, c, m * P:(m + 1) * P],
                            x_tiles[c][:, lo:lo + 512],
                            start=(c == 0), stop=(c == DC - 1))
                sl = out_sb[:, m, t2i * 1024:(t2i + 1) * 1024]
                if on_act:
                    nc.scalar.activation(sl, ps[:], AF.Copy, scale=float(sc_))
                else:
                    nc.vector.tensor_copy(sl, ps[:])

        def rv_block():
            for sc in range(SC):
                ps = psS.tile([P, 512], fp32, tag="ps", name="prv")
                for c in range(DC):
                    nc.tensor.matmul(
                        ps[:, :hs], xk[c][:, sc * P:(sc + 1) * P],
                        wv_sb[:, c], start=(c == 0), stop=(c == DC - 1))
                nc.vector.tensor_copy(
                    rv_sb[:, sc].rearrange("p (h c) -> p h c", c=adim + 1)[:, :, 0:adim],
                    ps[:, :hs].rearrange("p (h c) -> p h c", c=adim))

        # ---- middle-phase blocks ----
        # AV chunks from TWO units back are interleaved between score pairs:
        # their t is fully computed, so they never stall the in-order PE and
        # fill the relu-gated gaps between pairs (keeps the clock ramped).
        # They start at pair 2 so the po PSUM slot (freed by the previous
        # unit's oTnorm on DVE) is available by then.
        AV_SCHED = {2: (0, 4), 3: (4, 8), 4: (8, 12), 5: (12, 16)}

        def scores_block(h, qlo, u, avw=None):
            hp, hc = (h % 2) * adim, h // 2
            po = None
            n_av = 0
            if avw is not None:
                hprev, qloprev, tprev = avw
                po = psS.tile([P, 512], fp32, tag="ps", name="pav")
            for k in range(8):          # sc pairs
                if po is not None and k in AV_SCHED:
                    lo, hi = AV_SCHED[k]
                    for sc in range(lo, hi):
                        nc.tensor.matmul(
                            po[0:adim + 1, :],
                            rv_sb[:, sc, hprev * (adim + 1):(hprev + 1) * (adim + 1)],
                            tprev[:, sc], start=(n_av == 0), stop=(n_av == SC - 1))
                        n_av += 1
                ps = psB.tile([P, 1024], fp32, tag="pb", name="pscore")
                for j in (0, 1):
                    sc = 2 * k + j
                    nc.tensor.matmul(
                        ps[:, j * 512:(j + 1) * 512],
                        rkT_sb[hp:hp + adim, hc, sc * P:(sc + 1) * P],
                        rqT_sb[hp:hp + adim, hc, qlo:qlo + qt],
                        start=True, stop=True)
                up = u[:, 2 * k:2 * k + 2].rearrange("p a b -> p (a b)")
                nc.scalar.activation(up, ps[:], AF.Relu, bias=nb128[:])
            return po

        def ew_block(h, u, mblk):
            """Fully in place: u *= u (square), then u *= mask, in 4-sc
            chunks; u then IS t and feeds the AV matmuls two units later.
            GpSimd takes one chunk of each pass."""
            for j in range(4):
                u4 = u[:, 4 * j:4 * j + 4].rearrange("p a b -> p (a b)")
                eng = nc.gpsimd if j == 3 else nc.vector
                eng.tensor_mul(u4, u4, u4)
            for j in range(4):
                u4 = u[:, 4 * j:4 * j + 4].rearrange("p a b -> p (a b)")
                m4 = mblk[:, 4 * j:4 * j + 4].rearrange("p a b -> p (a b)")
                eng = nc.gpsimd if j == 3 else nc.vector
                eng.tensor_mul(u4, u4, m4)

        def av_mm(h, qlo, t):
            po = psS.tile([P, 512], fp32, tag="ps", name="pav")
            for sc in range(SC):
                nc.tensor.matmul(
                    po[0:adim + 1, :],
                    rv_sb[:, sc, h * (adim + 1):(h + 1) * (adim + 1)],
                    t[:, sc], start=(sc == 0), stop=(sc == SC - 1))
            return (h, qlo, po)

        def av_rec(h, qlo, po):
            """rec = 1/den on the ACT Reciprocal unit, via a raw instruction
            (the bass wrapper refuses it; its accuracy is amply within our
            2e-2 budget, and it shares an act table with Relu/Copy). The DVE
            reciprocal costs 3.4us per [1,512] row - far too slow."""
            rec = recp.tile([1, 512], fp32, tag="rec", name="rect")
            se = nc.scalar
            ins_l = [se.lower_ap(po[adim:adim + 1, :])]
            for arg in (0.0, 1.0, 0.0):     # bias, scale, alpha
                ins_l.append(mybir.ImmediateValue(dtype=fp32, value=arg))
            se.add_instruction(mybir.InstActivation(
                name=se.bass.get_next_instruction_name(),
                func=AF.Reciprocal, ins=ins_l, outs=[se.lower_ap(rec[:])]))
            return (h, qlo, po, rec)

        def av_fin(h, qlo, po, rec):
            """Broadcast rec via a K=1 PE outer product (GpSimd
            partition_broadcast forces Q7 library swaps costing ~7us/unit),
            copy it to SBUF (DVE reads at most one PSUM operand), then
            normalize + evict oT."""
            hp, hc = (h % 2) * adim, h // 2
            rb = psB.tile([P, 512], fp32, tag="pb", name="prb")
            nc.tensor.matmul(rb[0:adim, :], ones1[0:1, 0:adim], rec[:],
                             start=True, stop=True)
            rb_sb = rbp.tile([adim, 512], fp32, tag="rb", name="rbt")
            nc.vector.tensor_copy(rb_sb[:], rb[0:adim, :])
            nc.vector.tensor_mul(oT_sb[hp:hp + adim, hc, qlo:qlo + qt],
                                 po[0:adim, :], rb_sb[:])

        def outproj_qc(iq, qc):
            qlo = iq * qt
            ps = psB.tile([P, 1024], fp32, tag="pb", name="pout")
            for c in range(HC):
                for j in (0, 1):
                    nc.tensor.matmul(
                        ps[:, j * 512:(j + 1) * 512],
                        oT_sb[:, c, qlo + qc * P:qlo + (qc + 1) * P],
                        wo_sb[:, c, j * 512:(j + 1) * 512],
                        start=(c == 0), stop=(c == HC - 1))
            ob = outp.tile([P, 1024], bf16, tag="ob", name="obt")
            nc.vector.tensor_copy(ob[:], ps[:])
            nc.sync.dma_start(out_t[iq * (qt // P) + qc], ob[:])

        # ---- stage A ----
        proj_block(0, wq_sb, xq, rqT_sb, on_act=True, sc_=scale)
        proj_block(1, wq_sb, xq, rqT_sb, on_act=True, sc_=scale)
        proj_block(0, wk_sb, xk, rkT_sb, on_act=False)

        # ---- middle: units two-deep pipelined. Unit (iq,h): av_fin of the
        # unit three back; AV matmuls of the unit two back interleaved with
        # this unit's score pairs; reciprocal of the popped unit; in-place
        # elementwise of this unit. Out-projection of q tile iq-1 flushed at
        # h==3 (after its last av_fin). ----
        av_q = []             # (h, qlo, u) units awaiting AV matmuls
        fin = None            # av_rec result awaiting av_fin
        fin_h3, fin_iq = False, 0
        pending_out = []      # (iq, qc) out-projection chunks ready to emit
        for iq in range(NQ):
            qlo = iq * qt
            if iq == 0:
                mblk = mblk0
            else:
                mblk = maskp.tile([P, SC, qt], bf16, tag="m", name=f"mb{iq}")
                nc.sync.dma_start(mblk[:], maskT_t[:, :, qlo:qlo + qt])
            for h in range(hpg):
                u = upool.tile([P, SC, qt], bf16, tag="u", name="ut")
                if fin is not None:
                    av_fin(*fin)
                    fin = None
                    if fin_h3:
                        pending_out.extend((fin_iq, qc) for qc in range(4))
                if iq == 0 and h == 1:
                    rv_block()      # before the AV matmuls, which read rv_sb
                avw = av_q.pop(0) if len(av_q) >= 2 else None
                po = scores_block(h, qlo, u, avw=avw)
                if pending_out and h == 3:
                    while pending_out:
                        outproj_qc(*pending_out.pop(0))
                if iq == 0 and h == 0:
                    proj_block(1, wk_sb, xk, rkT_sb, on_act=False)
                if po is not None:
                    fin = av_rec(avw[0], avw[1], po)
                    fin_h3, fin_iq = (avw[0] == hpg - 1), avw[1] // qt
                ew_block(h, u, mblk)
                av_q.append((h, qlo, u))
        # tail: drain the two-deep pipeline
        if fin is not None:
            av_fin(*fin)
        for (hh, qq, uu) in av_q:
            _, _, po = av_mm(hh, qq, uu)
            f = av_rec(hh, qq, po)
            av_fin(*f)
        pending_out.extend((NQ - 1, qc) for qc in range(4))
        for w in pending_out:
            outproj_qc(*w)

    nc.compile()
    return nc


def _shard_inputs(iQ, iK, mask, Wq, Wkv, Wo, nbias):
    in_maps = []
    maskT_by_b = [np.ascontiguousarray((~mask[b]).T).astype(BF16)
                  for b in range(B)]
    qT_by_b = [np.ascontiguousarray(iQ[b].T).astype(BF16) for b in range(B)]
    kT_by_b = [np.ascontiguousarray(iK[b].T).astype(BF16) for b in range(B)]
    nb = np.asarray(nbias, np.float32).reshape(1, 1)
    for ci in range(N_CORES):
        b, g = ci // GROUPS, ci % GROUPS
        hsl = slice(g * HS, (g + 1) * HS)
        in_maps.append({
            "qT": qT_by_b[b],
            "kT": kT_by_b[b],
            "wqT": np.ascontiguousarray(Wq[hsl].T).astype(BF16),
            "wkT": np.ascontiguousarray(Wkv[hsl].T).astype(BF16),
            "wvT": np.ascontiguousarray(Wkv[HSIZE + g * HS:HSIZE + (g + 1) * HS].T).astype(BF16),
            "woT": np.ascontiguousarray(Wo[:, hsl].T).astype(BF16),
            "maskT": maskT_by_b[b],
            "nbias": nb,
        })
    return in_maps


def kernel(iQ, iK, mask, Wq, Wkv, Wo, nbias):
    global _COMPILED
    from concourse.bass_utils import run_bass_kernel_spmd

    if _COMPILED is None:
        _COMPILED = _build()
    in_maps = _shard_inputs(np.asarray(iQ, np.float32), np.asarray(iK, np.float32),
                            np.asarray(mask), np.asarray(Wq, np.float32),
                            np.asarray(Wkv, np.float32), np.asarray(Wo, np.float32),
                            np.asarray(nbias, np.float32))
    res = run_bass_kernel_spmd(_COMPILED, in_maps, list(range(N_CORES))).results
    out = np.zeros((B, Q, D), np.float32)
    for ci in range(N_CORES):
        out[ci // GROUPS] += np.asarray(res[ci]["out"], np.float32)
    return out


# revision 45
# speedup vs baseline: 1.3512x; 1.0028x over previous
"""Sparse cross-attention (squared-ReLU normalizer) on 8 TRN2 NeuronCores.

Sharding: 8 cores = batch(2) x head-group(4). Each core owns one batch and
4 of 16 heads (a 256-wide slice of hsize): Wq/Wkv column-parallel,
Wo row-parallel (partial outputs summed on host), mask replicated per
batch shard.

Per-core kernel, engine-balanced (bf16 matmuls, fp32 PSUM):
  stage A: rqT (hs,q), rkT (hs,s) via weight-stationary projections into
    [128,1024] PSUM tiles; rv (s, hs+ones) with kT chunks as weights.
    rq evicted on ACT (scale 1/sqrt(a) folded), rk/rv evicted on DVE.
    Input DMA issues are spread across engine queues (weights on DVE,
    mask on ACT, x on sync) so transfers overlap from t=0.
  middle, per unit = (q-tile 512, head): AV matmuls of the previous head
    run first (their t is ready - a contiguous PE burst), then one out-
    projection chunk, then 8 score-matmul pairs -> [128,1024] PSUM, relu
    +nbias on ACT -> u; square IN PLACE (u*=u) and mask mul (t=u*maskT)
    on DVE with the tail s-chunks on GpSimd. AV uses rv chunks as
    weights giving oT (64,q) + denominator row; 1/den via ACT exp(-ln);
    GpSimd partition-broadcast spreads rec; DVE normalizes+evicts oT.
  out projection: oT^T @ woT in PSUM, evicted bf16 (ACT/DVE alternating),
    DMA'd to DRAM; host sums the 4 row-parallel partials in fp32.
"""

import numpy as np
import ml_dtypes

BF16 = ml_dtypes.bfloat16

B, Q, S, D = 2, 2048, 2048, 1024
NUM_HEAD, ADIM = 16, 64
HSIZE = NUM_HEAD * ADIM
N_CORES = 8
GROUPS = 4                  # head groups (tensor-parallel dim)
HPG = NUM_HEAD // GROUPS    # 4 heads per core
HS = HPG * ADIM             # 256: per-core hsize slice
P = 128

POOL_SC = 3                 # trailing s-chunks per mul pass on GpSimd

_COMPILED = None


def _build(q=Q, s=S, d=D, hpg=HPG, adim=ADIM, qt=512):
    """Build + compile the per-core Bass program. Returns the Bacc."""
    from contextlib import ExitStack
    import concourse.bass as bass
    import concourse.mybir as mybir
    import concourse.tile as tile
    from concourse import bacc

    fp32 = mybir.dt.float32
    bf16 = mybir.dt.bfloat16
    AF = mybir.ActivationFunctionType

    hs = hpg * adim
    DC = d // P          # contraction chunks for projections (8)
    NQ = q // qt         # q tiles (4)
    SC = s // P          # s chunks (16)
    HC = hs // P         # hsize-slice chunks (2)
    DVE_SC = SC - POOL_SC
    assert hs % P == 0 and q % qt == 0 and qt == 512 and d == 1024

    nc = bacc.Bacc("TRN2", target_bir_lowering=False, debug=False,
                   num_devices=N_CORES)

    qT = nc.dram_tensor("qT", [d, q], bf16, kind="ExternalInput").ap()
    kT = nc.dram_tensor("kT", [d, s], bf16, kind="ExternalInput").ap()
    wqT = nc.dram_tensor("wqT", [d, hs], bf16, kind="ExternalInput").ap()
    wkT = nc.dram_tensor("wkT", [d, hs], bf16, kind="ExternalInput").ap()
    wvT = nc.dram_tensor("wvT", [d, hs], bf16, kind="ExternalInput").ap()
    woT = nc.dram_tensor("woT", [hs, d], bf16, kind="ExternalInput").ap()
    maskT = nc.dram_tensor("maskT", [s, q], bf16, kind="ExternalInput").ap()
    nbias = nc.dram_tensor("nbias", [1, 1], fp32, kind="ExternalInput").ap()
    out = nc.dram_tensor("out", [q, d], bf16, kind="ExternalOutput").ap()

    qT_t = qT.rearrange("(c p) q -> c p q", p=P)        # [8, 128, q]
    kT_t = kT.rearrange("(c p) s -> c p s", p=P)
    wqT_t = wqT.rearrange("(c p) h -> c p h", p=P)
    wkT_t = wkT.rearrange("(c p) h -> c p h", p=P)
    wvT_t = wvT.rearrange("(c p) h -> c p h", p=P)
    woT_t = woT.rearrange("(c p) d -> c p d", p=P)      # [2, 128, d]
    maskT_t = maskT.rearrange("(c p) q -> p c q", p=P)  # [128, SC, q]
    out_t = out.rearrange("(c p) d -> c p d", p=P)      # [q/P, 128, d]

    scale = 1.0 / np.sqrt(np.float32(adim))

    with tile.TileContext(nc) as tc, ExitStack() as ctx:
        const = ctx.enter_context(tc.tile_pool(name="const", bufs=1))
        wpool = ctx.enter_context(tc.tile_pool(name="w", bufs=1))
        xpool = ctx.enter_context(tc.tile_pool(name="x", bufs=9))
        actp = ctx.enter_context(tc.tile_pool(name="act", bufs=1))
        maskp = ctx.enter_context(tc.tile_pool(name="mask", bufs=2))
        upool = ctx.enter_context(tc.tile_pool(name="u", bufs=3))
        recp = ctx.enter_context(tc.tile_pool(name="rec", bufs=3))
        rbp = ctx.enter_context(tc.tile_pool(name="rb", bufs=2))
        outp = ctx.enter_context(tc.tile_pool(name="out", bufs=2))
        psB = ctx.enter_context(tc.tile_pool(name="psB", bufs=3, space="PSUM"))
        psS = ctx.enter_context(tc.tile_pool(name="psS", bufs=2, space="PSUM"))

        # ---- constants ----
        ones1 = const.tile([1, P], fp32)
        nc.any.memset(ones1[:], 1.0)
        nb1 = const.tile([1, 1], fp32)
        nc.sync.dma_start(nb1[:], nbias[:])
        ps_nb = psS.tile([P, 512], fp32, tag="ps", name="psnb")
        nc.tensor.matmul(ps_nb[:, 0:1], ones1[:], nb1[:], start=True, stop=True)
        nb128 = const.tile([P, 1], fp32)
        nc.scalar.copy(nb128[:], ps_nb[:, 0:1])

        # ---- resident weights on the ACT DMA queue so the sync queue is
        # free for the x chunks from t=0 ----
        wq_sb = wpool.tile([P, DC, hs], bf16)
        wk_sb = wpool.tile([P, DC, hs], bf16)
        wv_sb = wpool.tile([P, DC, hs], bf16)
        wo_sb = wpool.tile([P, HC, d], bf16)
        # ---- resident activations ----
        rqT_sb = actp.tile([P, HC, q], bf16)    # (hs, q)
        rkT_sb = actp.tile([P, HC, s], bf16)    # (hs, s)
        rv_sb = actp.tile([P, SC, hpg * (adim + 1)], bf16)  # (s, hs + ones)
        oT_sb = actp.tile([P, HC, q], bf16)     # (hs, q)
        nc.gpsimd.memset(rv_sb[:], 1.0)         # ones cols survive at 64::65

        # ---- stage A inputs; mask for the first q tile is hoisted before
        # the K-side loads; xk reuses xq buffers so its DMAs stream in as
        # Qm1 frees them chunk by chunk ----
        xq = []
        for c in range(DC):
            nc.sync.dma_start(wq_sb[:, c], wqT_t[c])
            xt = xpool.tile([P, q], bf16, tag="x", name=f"xq{c}")
            nc.sync.dma_start(xt[:], qT_t[c])
            xq.append(xt)
        for c in range(DC):
            nc.sync.dma_start(wk_sb[:, c], wkT_t[c])
            nc.sync.dma_start(wv_sb[:, c], wvT_t[c])
        xk = []
        for c in range(DC):
            xt = xpool.tile([P, s], bf16, tag="x", name=f"xk{c}")
            nc.sync.dma_start(xt[:], kT_t[c])
            xk.append(xt)
        mblk0 = maskp.tile([P, SC, qt], bf16, tag="m", name="mb0")
        nc.sync.dma_start(mblk0[:], maskT_t[:, :, 0:qt])
        for c in range(HC):
            nc.sync.dma_start(wo_sb[:, c], woT_t[c])

        def proj_block(m, w_sb, x_tiles, out_sb, on_act, sc_=1.0):
            """out_sb[:, m, :] = (W_m @ X); matmul N capped at one PSUM bank."""
            for t2i in range(q // 1024):
                ps = psB.tile([P, 1024], fp32, tag="pb", name="pproj")
                for c in range(DC):
                    for j in (0, 1):
                        lo = t2i * 1024 + j * 512
                        nc.tensor.matmul(
                            ps[:, j * 512:(j + 1) * 512],
                            w_sb[:, c, m * P:(m + 1) * P],
                            x_tiles[c][:, lo:lo + 512],
                            start=(c == 0), stop=(c == DC - 1))
                sl = out_sb[:, m, t2i * 1024:(t2i + 1) * 1024]
                if on_act:
                    nc.scalar.activation(sl, ps[:], AF.Copy, scale=float(sc_))
                else:
                    nc.vector.tensor_copy(sl, ps[:])

        def rv_block():
            for sc in range(SC):
                ps = psS.tile([P, 512], fp32, tag="ps", name="prv")
                for c in range(DC):
                    nc.tensor.matmul(
                        ps[:, :hs], xk[c][:, sc * P:(sc + 1) * P],
                        wv_sb[:, c], start=(c == 0), stop=(c == DC - 1))
                nc.vector.tensor_copy(
                    rv_sb[:, sc].rearrange("p (h c) -> p h c", c=adim + 1)[:, :, 0:adim],
                    ps[:, :hs].rearrange("p (h c) -> p h c", c=adim))

        # ---- middle-phase blocks ----
        # AV chunks from TWO units back are interleaved between score pairs:
        # their t is fully computed, so they never stall the in-order PE and
        # fill the relu-gated gaps between pairs (keeps the clock ramped).
        # They start at pair 2 so the po PSUM slot (freed by the previous
        # unit's oTnorm on DVE) is available by then.
        AV_SCHED = {2: (0, 4), 3: (4, 8), 4: (8, 12), 5: (12, 16)}

        def rv_part(lo, hi):
            """rv projection for s-chunks [lo,hi) - filler PE work emitted
            into the AV slots of the first two units (no AV work there)."""
            for sc in range(lo, hi):
                ps = psS.tile([P, 512], fp32, tag="ps", name="prv")
                for c in range(DC):
                    nc.tensor.matmul(
                        ps[:, :hs], xk[c][:, sc * P:(sc + 1) * P],
                        wv_sb[:, c], start=(c == 0), stop=(c == DC - 1))
                nc.vector.tensor_copy(
                    rv_sb[:, sc].rearrange("p (h c) -> p h c", c=adim + 1)[:, :, 0:adim],
                    ps[:, :hs].rearrange("p (h c) -> p h c", c=adim))

        def scores_block(h, qlo, u, avw=None, filler=None):
            hp, hc = (h % 2) * adim, h // 2
            po = None
            n_av = 0
            if avw is not None:
                hprev, qloprev, tprev = avw
                po = psS.tile([P, 512], fp32, tag="ps", name="pav")
            for k in range(8):          # sc pairs
                if po is not None and k in AV_SCHED:
                    lo, hi = AV_SCHED[k]
                    for sc in range(lo, hi):
                        nc.tensor.matmul(
                            po[0:adim + 1, :],
                            rv_sb[:, sc, hprev * (adim + 1):(hprev + 1) * (adim + 1)],
                            tprev[:, sc], start=(n_av == 0), stop=(n_av == SC - 1))
                        n_av += 1
                elif filler is not None and k in AV_SCHED:
                    base = filler + 2 * (k - 2)
                    rv_part(base, base + 2)
                ps = psB.tile([P, 1024], fp32, tag="pb", name="pscore")
                for j in (0, 1):
                    sc = 2 * k + j
                    nc.tensor.matmul(
                        ps[:, j * 512:(j + 1) * 512],
                        rkT_sb[hp:hp + adim, hc, sc * P:(sc + 1) * P],
                        rqT_sb[hp:hp + adim, hc, qlo:qlo + qt],
                        start=True, stop=True)
                up = u[:, 2 * k:2 * k + 2].rearrange("p a b -> p (a b)")
                nc.scalar.activation(up, ps[:], AF.Relu, bias=nb128[:])
            return po

        def ew_block(h, u, mblk):
            """Fully in place: u *= u (square), then u *= mask, in 4-sc
            chunks; u then IS t and feeds the AV matmuls two units later.
            GpSimd takes one chunk of each pass."""
            for j in range(4):
                u4 = u[:, 4 * j:4 * j + 4].rearrange("p a b -> p (a b)")
                eng = nc.gpsimd if j == 3 else nc.vector
                eng.tensor_mul(u4, u4, u4)
            for j in range(4):
                u4 = u[:, 4 * j:4 * j + 4].rearrange("p a b -> p (a b)")
                m4 = mblk[:, 4 * j:4 * j + 4].rearrange("p a b -> p (a b)")
                eng = nc.gpsimd if j == 3 else nc.vector
                eng.tensor_mul(u4, u4, m4)

        def av_mm(h, qlo, t):
            po = psS.tile([P, 512], fp32, tag="ps", name="pav")
            for sc in range(SC):
                nc.tensor.matmul(
                    po[0:adim + 1, :],
                    rv_sb[:, sc, h * (adim + 1):(h + 1) * (adim + 1)],
                    t[:, sc], start=(sc == 0), stop=(sc == SC - 1))
            return (h, qlo, po)

        def av_rec(h, qlo, po):
            """rec = 1/den on the ACT Reciprocal unit, via a raw instruction
            (the bass wrapper refuses it; its accuracy is amply within our
            2e-2 budget, and it shares an act table with Relu/Copy). The DVE
            reciprocal costs 3.4us per [1,512] row - far too slow."""
            rec = recp.tile([1, 512], fp32, tag="rec", name="rect")
            se = nc.scalar
            ins_l = [se.lower_ap(po[adim:adim + 1, :])]
            for arg in (0.0, 1.0, 0.0):     # bias, scale, alpha
                ins_l.append(mybir.ImmediateValue(dtype=fp32, value=arg))
            se.add_instruction(mybir.InstActivation(
                name=se.bass.get_next_instruction_name(),
                func=AF.Reciprocal, ins=ins_l, outs=[se.lower_ap(rec[:])]))
            return (h, qlo, po, rec)

        def av_fin(h, qlo, po, rec):
            """Broadcast rec via a K=1 PE outer product (GpSimd
            partition_broadcast forces Q7 library swaps costing ~7us/unit),
            copy it to SBUF (DVE reads at most one PSUM operand), then
            normalize + evict oT."""
            hp, hc = (h % 2) * adim, h // 2
            rb = psB.tile([P, 512], fp32, tag="pb", name="prb")
            nc.tensor.matmul(rb[0:adim, :], ones1[0:1, 0:adim], rec[:],
                             start=True, stop=True)
            rb_sb = rbp.tile([adim, 512], fp32, tag="rb", name="rbt")
            nc.vector.tensor_copy(rb_sb[:], rb[0:adim, :])
            nc.vector.tensor_mul(oT_sb[hp:hp + adim, hc, qlo:qlo + qt],
                                 po[0:adim, :], rb_sb[:])

        def outproj_qc(iq, qc):
            qlo = iq * qt
            ps = psB.tile([P, 1024], fp32, tag="pb", name="pout")
            for c in range(HC):
                for j in (0, 1):
                    nc.tensor.matmul(
                        ps[:, j * 512:(j + 1) * 512],
                        oT_sb[:, c, qlo + qc * P:qlo + (qc + 1) * P],
                        wo_sb[:, c, j * 512:(j + 1) * 512],
                        start=(c == 0), stop=(c == HC - 1))
            ob = outp.tile([P, 1024], bf16, tag="ob", name="obt")
            if qc % 2 == 0:
                nc.scalar.copy(ob[:], ps[:])
            else:
                nc.vector.tensor_copy(ob[:], ps[:])
            nc.sync.dma_start(out_t[iq * (qt // P) + qc], ob[:])

        # ---- stage A: all four projections back to back (continuous PE
        # ramp); the rv projection is deferred into the first two units'
        # empty AV slots so the middle phase starts ~14us earlier ----
        proj_block(0, wq_sb, xq, rqT_sb, on_act=True, sc_=scale)
        proj_block(1, wq_sb, xq, rqT_sb, on_act=True, sc_=scale)
        proj_block(0, wk_sb, xk, rkT_sb, on_act=False)
        proj_block(1, wk_sb, xk, rkT_sb, on_act=False)

        # ---- middle: units two-deep pipelined. Unit (iq,h): av_fin of the
        # unit three back; AV matmuls of the unit two back interleaved with
        # this unit's score pairs; reciprocal of the popped unit; in-place
        # elementwise of this unit. Out-projection of q tile iq-1 flushed at
        # h==3 (after its last av_fin). ----
        av_q = []             # (h, qlo, u) units awaiting AV matmuls
        fin = None            # av_rec result awaiting av_fin
        fin_h3, fin_iq = False, 0
        pending_out = []      # (iq, qc) out-projection chunks ready to emit
        for iq in range(NQ):
            qlo = iq * qt
            if iq == 0:
                mblk = mblk0
            else:
                mblk = maskp.tile([P, SC, qt], bf16, tag="m", name=f"mb{iq}")
                nc.sync.dma_start(mblk[:], maskT_t[:, :, qlo:qlo + qt])
            for h in range(hpg):
                u = upool.tile([P, SC, qt], bf16, tag="u", name="ut")
                if fin is not None:
                    av_fin(*fin)
                    fin = None
                    if fin_h3:
                        pending_out.extend((fin_iq, qc) for qc in range(4))
                avw = av_q.pop(0) if len(av_q) >= 2 else None
                filler = 8 * h if iq == 0 and h < 2 else None
                po = scores_block(h, qlo, u, avw=avw, filler=filler)
                if pending_out and h == 3:
                    while pending_out:
                        outproj_qc(*pending_out.pop(0))
                if po is not None:
                    fin = av_rec(avw[0], avw[1], po)
                    fin_h3, fin_iq = (avw[0] == hpg - 1), avw[1] // qt
                ew_block(h, u, mblk)
                av_q.append((h, qlo, u))
        # tail: drain the two-deep pipeline
        if fin is not None:
            av_fin(*fin)
        for (hh, qq, uu) in av_q:
            _, _, po = av_mm(hh, qq, uu)
            f = av_rec(hh, qq, po)
            av_fin(*f)
        pending_out.extend((NQ - 1, qc) for qc in range(4))
        for w in pending_out:
            outproj_qc(*w)

    nc.compile()
    return nc


def _shard_inputs(iQ, iK, mask, Wq, Wkv, Wo, nbias):
    in_maps = []
    maskT_by_b = [np.ascontiguousarray((~mask[b]).T).astype(BF16)
                  for b in range(B)]
    qT_by_b = [np.ascontiguousarray(iQ[b].T).astype(BF16) for b in range(B)]
    kT_by_b = [np.ascontiguousarray(iK[b].T).astype(BF16) for b in range(B)]
    nb = np.asarray(nbias, np.float32).reshape(1, 1)
    for ci in range(N_CORES):
        b, g = ci // GROUPS, ci % GROUPS
        hsl = slice(g * HS, (g + 1) * HS)
        in_maps.append({
            "qT": qT_by_b[b],
            "kT": kT_by_b[b],
            "wqT": np.ascontiguousarray(Wq[hsl].T).astype(BF16),
            "wkT": np.ascontiguousarray(Wkv[hsl].T).astype(BF16),
            "wvT": np.ascontiguousarray(Wkv[HSIZE + g * HS:HSIZE + (g + 1) * HS].T).astype(BF16),
            "woT": np.ascontiguousarray(Wo[:, hsl].T).astype(BF16),
            "maskT": maskT_by_b[b],
            "nbias": nb,
        })
    return in_maps


def kernel(iQ, iK, mask, Wq, Wkv, Wo, nbias):
    global _COMPILED
    from concourse.bass_utils import run_bass_kernel_spmd

    if _COMPILED is None:
        _COMPILED = _build()
    in_maps = _shard_inputs(np.asarray(iQ, np.float32), np.asarray(iK, np.float32),
                            np.asarray(mask), np.asarray(Wq, np.float32),
                            np.asarray(Wkv, np.float32), np.asarray(Wo, np.float32),
                            np.asarray(nbias, np.float32))
    res = run_bass_kernel_spmd(_COMPILED, in_maps, list(range(N_CORES))).results
    out = np.zeros((B, Q, D), np.float32)
    for ci in range(N_CORES):
        out[ci // GROUPS] += np.asarray(res[ci]["out"], np.float32)
    return out


# revision 47
# speedup vs baseline: 1.4066x; 1.0410x over previous
"""Sparse cross-attention (squared-ReLU normalizer) on 8 TRN2 NeuronCores.

Sharding: 8 cores = batch(2) x head-group(4). Each core owns one batch and
4 of 16 heads (a 256-wide slice of hsize): Wq/Wkv column-parallel,
Wo row-parallel (partial outputs summed on host), mask replicated per
batch shard.

Per-core kernel, engine-balanced (bf16 matmuls, fp32 PSUM):
  stage A: rqT (hs,q), rkT (hs,s) via weight-stationary projections into
    [128,1024] PSUM tiles; rv (s, hs+ones) with kT chunks as weights.
    rq evicted on ACT (scale 1/sqrt(a) folded), rk/rv evicted on DVE.
    Input DMA issues are spread across engine queues (weights on DVE,
    mask on ACT, x on sync) so transfers overlap from t=0.
  middle, per unit = (q-tile 512, head): AV matmuls of the previous head
    run first (their t is ready - a contiguous PE burst), then one out-
    projection chunk, then 8 score-matmul pairs -> [128,1024] PSUM, relu
    +nbias on ACT -> u; square IN PLACE (u*=u) and mask mul (t=u*maskT)
    on DVE with the tail s-chunks on GpSimd. AV uses rv chunks as
    weights giving oT (64,q) + denominator row; 1/den via ACT exp(-ln);
    GpSimd partition-broadcast spreads rec; DVE normalizes+evicts oT.
  out projection: oT^T @ woT in PSUM, evicted bf16 (ACT/DVE alternating),
    DMA'd to DRAM; host sums the 4 row-parallel partials in fp32.
"""

import numpy as np
import ml_dtypes

BF16 = ml_dtypes.bfloat16

B, Q, S, D = 2, 2048, 2048, 1024
NUM_HEAD, ADIM = 16, 64
HSIZE = NUM_HEAD * ADIM
N_CORES = 8
GROUPS = 4                  # head groups (tensor-parallel dim)
HPG = NUM_HEAD // GROUPS    # 4 heads per core
HS = HPG * ADIM             # 256: per-core hsize slice
P = 128

POOL_SC = 3                 # trailing s-chunks per mul pass on GpSimd

_COMPILED = None


def _build(q=Q, s=S, d=D, hpg=HPG, adim=ADIM, qt=512):
    """Build + compile the per-core Bass program. Returns the Bacc."""
    from contextlib import ExitStack
    import concourse.bass as bass
    import concourse.mybir as mybir
    import concourse.tile as tile
    from concourse import bacc

    fp32 = mybir.dt.float32
    bf16 = mybir.dt.bfloat16
    AF = mybir.ActivationFunctionType

    hs = hpg * adim
    DC = d // P          # contraction chunks for projections (8)
    NQ = q // qt         # q tiles (4)
    SC = s // P          # s chunks (16)
    HC = hs // P         # hsize-slice chunks (2)
    DVE_SC = SC - POOL_SC
    assert hs % P == 0 and q % qt == 0 and qt == 512 and d == 1024

    nc = bacc.Bacc("TRN2", target_bir_lowering=False, debug=False,
                   num_devices=N_CORES)

    qT = nc.dram_tensor("qT", [d, q], bf16, kind="ExternalInput").ap()
    kT = nc.dram_tensor("kT", [d, s], bf16, kind="ExternalInput").ap()
    wqT = nc.dram_tensor("wqT", [d, hs], bf16, kind="ExternalInput").ap()
    wkT = nc.dram_tensor("wkT", [d, hs], bf16, kind="ExternalInput").ap()
    wvT = nc.dram_tensor("wvT", [d, hs], bf16, kind="ExternalInput").ap()
    woT = nc.dram_tensor("woT", [hs, d], bf16, kind="ExternalInput").ap()
    maskT = nc.dram_tensor("maskT", [s, q], bf16, kind="ExternalInput").ap()
    nbias = nc.dram_tensor("nbias", [1, 1], fp32, kind="ExternalInput").ap()
    out = nc.dram_tensor("out", [q, d], bf16, kind="ExternalOutput").ap()

    qT_t = qT.rearrange("(c p) q -> c p q", p=P)        # [8, 128, q]
    kT_t = kT.rearrange("(c p) s -> c p s", p=P)
    wqT_t = wqT.rearrange("(c p) h -> c p h", p=P)
    wkT_t = wkT.rearrange("(c p) h -> c p h", p=P)
    wvT_t = wvT.rearrange("(c p) h -> c p h", p=P)
    woT_t = woT.rearrange("(c p) d -> c p d", p=P)      # [2, 128, d]
    maskT_t = maskT.rearrange("(c p) q -> p c q", p=P)  # [128, SC, q]
    out_t = out.rearrange("(c p) d -> c p d", p=P)      # [q/P, 128, d]

    scale = 1.0 / np.sqrt(np.float32(adim))

    with tile.TileContext(nc) as tc, ExitStack() as ctx:
        const = ctx.enter_context(tc.tile_pool(name="const", bufs=1))
        wpool = ctx.enter_context(tc.tile_pool(name="w", bufs=1))
        xpool = ctx.enter_context(tc.tile_pool(name="x", bufs=9))
        actp = ctx.enter_context(tc.tile_pool(name="act", bufs=1))
        maskp = ctx.enter_context(tc.tile_pool(name="mask", bufs=2))
        upool = ctx.enter_context(tc.tile_pool(name="u", bufs=3))
        recp = ctx.enter_context(tc.tile_pool(name="rec", bufs=3))
        rbp = ctx.enter_context(tc.tile_pool(name="rb", bufs=2))
        outp = ctx.enter_context(tc.tile_pool(name="out", bufs=2))
        psB = ctx.enter_context(tc.tile_pool(name="psB", bufs=3, space="PSUM"))
        psS = ctx.enter_context(tc.tile_pool(name="psS", bufs=2, space="PSUM"))

        # ---- constants ----
        ones1 = const.tile([1, P], bf16)
        nc.any.memset(ones1[:], 1.0)
        ones1f = const.tile([1, P], fp32)
        nc.any.memset(ones1f[:], 1.0)
        nb1 = const.tile([1, 1], fp32)
        nc.sync.dma_start(nb1[:], nbias[:])
        ps_nb = psS.tile([P, 512], fp32, tag="ps", name="psnb")
        nc.tensor.matmul(ps_nb[:, 0:1], ones1f[:], nb1[:], start=True, stop=True)
        nb128 = const.tile([P, 1], fp32)
        nc.scalar.copy(nb128[:], ps_nb[:, 0:1])

        # ---- resident weights on the ACT DMA queue so the sync queue is
        # free for the x chunks from t=0 ----
        wq_sb = wpool.tile([P, DC, hs], bf16)
        wk_sb = wpool.tile([P, DC, hs], bf16)
        wv_sb = wpool.tile([P, DC, hs], bf16)
        wo_sb = wpool.tile([P, HC, d], bf16)
        # ---- resident activations ----
        rqT_sb = actp.tile([P, HC, q], bf16)    # (hs, q)
        # rk zero-padded to K=128 per head: scores matmuls then use the full
        # PE array (K=64 row-group mode costs ~1.6x per row); the rhs is the
        # full two-head rqT chunk - the other head's rows hit zero weights.
        rkT_sb = actp.tile([P, hpg, s], bf16)   # (128, head, s)
        rv_sb = actp.tile([P, SC, hpg * (adim + 1)], bf16)  # (s, hs + ones)
        oT_sb = actp.tile([P, HC, q], bf16)     # (hs, q)
        nc.gpsimd.memset(rv_sb[:], 1.0)         # ones cols survive at 64::65
        nc.gpsimd.memset(rkT_sb[:], 0.0)        # zero rows for the K pad

        # ---- stage A inputs; mask for the first q tile is hoisted before
        # the K-side loads; xk reuses xq buffers so its DMAs stream in as
        # Qm1 frees them chunk by chunk ----
        xq = []
        for c in range(DC):
            nc.sync.dma_start(wq_sb[:, c], wqT_t[c])
            xt = xpool.tile([P, q], bf16, tag="x", name=f"xq{c}")
            nc.sync.dma_start(xt[:], qT_t[c])
            xq.append(xt)
        for c in range(DC):
            nc.sync.dma_start(wk_sb[:, c], wkT_t[c])
            nc.sync.dma_start(wv_sb[:, c], wvT_t[c])
        xk = []
        for c in range(DC):
            xt = xpool.tile([P, s], bf16, tag="x", name=f"xk{c}")
            nc.sync.dma_start(xt[:], kT_t[c])
            xk.append(xt)
        mblk0 = maskp.tile([P, SC, qt], bf16, tag="m", name="mb0")
        nc.sync.dma_start(mblk0[:], maskT_t[:, :, 0:qt])
        for c in range(HC):
            nc.sync.dma_start(wo_sb[:, c], woT_t[c])

        def proj_block(m, w_sb, x_tiles, out_sb, on_act, sc_=1.0, pad=False):
            """out_sb[:, m, :] = (W_m @ X); matmul N capped at one PSUM bank.
            pad=True scatters the two head-halves into the K=128-padded rk
            layout (row half (h%2)*64 of plane h)."""
            for t2i in range(q // 1024):
                ps = psB.tile([P, 1024], fp32, tag="pb", name="pproj")
                for c in range(DC):
                    for j in (0, 1):
                        lo = t2i * 1024 + j * 512
                        nc.tensor.matmul(
                            ps[:, j * 512:(j + 1) * 512],
                            w_sb[:, c, m * P:(m + 1) * P],
                            x_tiles[c][:, lo:lo + 512],
                            start=(c == 0), stop=(c == DC - 1))
                span = slice(t2i * 1024, (t2i + 1) * 1024)
                if pad:
                    nc.vector.tensor_copy(out_sb[0:adim, 2 * m, span],
                                          ps[0:adim, :])
                    nc.vector.tensor_copy(out_sb[adim:P, 2 * m + 1, span],
                                          ps[adim:P, :])
                elif on_act:
                    nc.scalar.activation(out_sb[:, m, span], ps[:],
                                         AF.Copy, scale=float(sc_))
                else:
                    nc.vector.tensor_copy(out_sb[:, m, span], ps[:])

        def rv_block():
            for sc in range(SC):
                ps = psS.tile([P, 512], fp32, tag="ps", name="prv")
                for c in range(DC):
                    nc.tensor.matmul(
                        ps[:, :hs], xk[c][:, sc * P:(sc + 1) * P],
                        wv_sb[:, c], start=(c == 0), stop=(c == DC - 1))
                nc.vector.tensor_copy(
                    rv_sb[:, sc].rearrange("p (h c) -> p h c", c=adim + 1)[:, :, 0:adim],
                    ps[:, :hs].rearrange("p (h c) -> p h c", c=adim))

        # ---- middle-phase blocks ----
        # AV chunks from TWO units back are interleaved between score pairs:
        # their t is fully computed, so they never stall the in-order PE and
        # fill the relu-gated gaps between pairs (keeps the clock ramped).
        # They start at pair 2 so the po PSUM slot (freed by the previous
        # unit's oTnorm on DVE) is available by then.
        AV_SCHED = {2: (0, 4), 3: (4, 8), 4: (8, 12), 5: (12, 16)}

        def rv_part(lo, hi):
            """rv projection for s-chunks [lo,hi) - filler PE work emitted
            into the AV slots of the first two units (no AV work there)."""
            for sc in range(lo, hi):
                ps = psS.tile([P, 512], fp32, tag="ps", name="prv")
                for c in range(DC):
                    nc.tensor.matmul(
                        ps[:, :hs], xk[c][:, sc * P:(sc + 1) * P],
                        wv_sb[:, c], start=(c == 0), stop=(c == DC - 1))
                nc.vector.tensor_copy(
                    rv_sb[:, sc].rearrange("p (h c) -> p h c", c=adim + 1)[:, :, 0:adim],
                    ps[:, :hs].rearrange("p (h c) -> p h c", c=adim))

        def scores_block(h, qlo, u, avw=None, filler=None):
            hp, hc = (h % 2) * adim, h // 2
            po = None
            n_av = 0
            if avw is not None:
                hprev, qloprev, tprev = avw
                po = psS.tile([P, 512], fp32, tag="ps", name="pav")
            for k in range(8):          # sc pairs
                if po is not None and k in AV_SCHED:
                    lo, hi = AV_SCHED[k]
                    for sc in range(lo, hi):
                        nc.tensor.matmul(
                            po[0:adim + 1, :],
                            rv_sb[:, sc, hprev * (adim + 1):(hprev + 1) * (adim + 1)],
                            tprev[:, sc], start=(n_av == 0), stop=(n_av == SC - 1))
                        n_av += 1
                elif filler is not None and k in AV_SCHED:
                    base = filler + 2 * (k - 2)
                    rv_part(base, base + 2)
                ps = psB.tile([P, 1024], fp32, tag="pb", name="pscore")
                for j in (0, 1):
                    sc = 2 * k + j
                    nc.tensor.matmul(
                        ps[:, j * 512:(j + 1) * 512],
                        rkT_sb[:, h, sc * P:(sc + 1) * P],
                        rqT_sb[:, hc, qlo:qlo + qt],
                        start=True, stop=True)
                up = u[:, 2 * k:2 * k + 2].rearrange("p a b -> p (a b)")
                nc.scalar.activation(up, ps[:], AF.Relu, bias=nb128[:])
            return po

        def ew_block(h, u, mblk):
            """Fully in place: u *= u (square), then u *= mask, in 4-sc
            chunks; u then IS t and feeds the AV matmuls two units later.
            GpSimd takes one chunk of each pass."""
            for j in range(4):
                u4 = u[:, 4 * j:4 * j + 4].rearrange("p a b -> p (a b)")
                eng = nc.gpsimd if j == 3 else nc.vector
                eng.tensor_mul(u4, u4, u4)
            for j in range(4):
                u4 = u[:, 4 * j:4 * j + 4].rearrange("p a b -> p (a b)")
                m4 = mblk[:, 4 * j:4 * j + 4].rearrange("p a b -> p (a b)")
                eng = nc.gpsimd if j == 3 else nc.vector
                eng.tensor_mul(u4, u4, m4)

        def av_mm(h, qlo, t):
            po = psS.tile([P, 512], fp32, tag="ps", name="pav")
            for sc in range(SC):
                nc.tensor.matmul(
                    po[0:adim + 1, :],
                    rv_sb[:, sc, h * (adim + 1):(h + 1) * (adim + 1)],
                    t[:, sc], start=(sc == 0), stop=(sc == SC - 1))
            return (h, qlo, po)

        def av_rec(h, qlo, po):
            """rec = 1/den on the ACT Reciprocal unit, via a raw instruction
            (the bass wrapper refuses it; its accuracy is amply within our
            2e-2 budget, and it shares an act table with Relu/Copy). The DVE
            reciprocal costs 3.4us per [1,512] row - far too slow."""
            rec = recp.tile([1, 512], bf16, tag="rec", name="rect")
            se = nc.scalar
            ins_l = [se.lower_ap(po[adim:adim + 1, :])]
            for arg in (0.0, 1.0, 0.0):     # bias, scale, alpha
                ins_l.append(mybir.ImmediateValue(dtype=fp32, value=arg))
            se.add_instruction(mybir.InstActivation(
                name=se.bass.get_next_instruction_name(),
                func=AF.Reciprocal, ins=ins_l, outs=[se.lower_ap(rec[:])]))
            return (h, qlo, po, rec)

        def av_fin(h, qlo, po, rec):
            """Broadcast rec via a K=1 PE outer product (GpSimd
            partition_broadcast forces Q7 library swaps costing ~7us/unit),
            copy it to SBUF (DVE reads at most one PSUM operand), then
            normalize + evict oT."""
            hp, hc = (h % 2) * adim, h // 2
            rb = psB.tile([P, 512], fp32, tag="pb", name="prb")
            nc.tensor.matmul(rb[0:adim, :], ones1[0:1, 0:adim], rec[:],
                             start=True, stop=True)
            rb_sb = rbp.tile([adim, 512], fp32, tag="rb", name="rbt")
            nc.vector.tensor_copy(rb_sb[:], rb[0:adim, :])
            nc.vector.tensor_mul(oT_sb[hp:hp + adim, hc, qlo:qlo + qt],
                                 po[0:adim, :], rb_sb[:])

        def outproj_qc(iq, qc):
            qlo = iq * qt
            ps = psB.tile([P, 1024], fp32, tag="pb", name="pout")
            for c in range(HC):
                for j in (0, 1):
                    nc.tensor.matmul(
                        ps[:, j * 512:(j + 1) * 512],
                        oT_sb[:, c, qlo + qc * P:qlo + (qc + 1) * P],
                        wo_sb[:, c, j * 512:(j + 1) * 512],
                        start=(c == 0), stop=(c == HC - 1))
            ob = outp.tile([P, 1024], bf16, tag="ob", name="obt")
            if qc % 2 == 0:
                nc.scalar.copy(ob[:], ps[:])
            else:
                nc.vector.tensor_copy(ob[:], ps[:])
            nc.sync.dma_start(out_t[iq * (qt // P) + qc], ob[:])

        # ---- stage A: all four projections back to back (continuous PE
        # ramp); the rv projection is deferred into the first two units'
        # empty AV slots so the middle phase starts ~14us earlier ----
        proj_block(0, wq_sb, xq, rqT_sb, on_act=True, sc_=scale)
        proj_block(1, wq_sb, xq, rqT_sb, on_act=True, sc_=scale)
        proj_block(0, wk_sb, xk, rkT_sb, on_act=False, pad=True)
        proj_block(1, wk_sb, xk, rkT_sb, on_act=False, pad=True)

        # ---- middle: units two-deep pipelined. Unit (iq,h): av_fin of the
        # unit three back; AV matmuls of the unit two back interleaved with
        # this unit's score pairs; reciprocal of the popped unit; in-place
        # elementwise of this unit. Out-projection of q tile iq-1 flushed at
        # h==3 (after its last av_fin). ----
        av_q = []             # (h, qlo, u) units awaiting AV matmuls
        fin = None            # av_rec result awaiting av_fin
        fin_h3, fin_iq = False, 0
        pending_out = []      # (iq, qc) out-projection chunks ready to emit
        for iq in range(NQ):
            qlo = iq * qt
            if iq == 0:
                mblk = mblk0
            else:
                mblk = maskp.tile([P, SC, qt], bf16, tag="m", name=f"mb{iq}")
                nc.sync.dma_start(mblk[:], maskT_t[:, :, qlo:qlo + qt])
            for h in range(hpg):
                u = upool.tile([P, SC, qt], bf16, tag="u", name="ut")
                if fin is not None:
                    av_fin(*fin)
                    fin = None
                    if fin_h3:
                        pending_out.extend((fin_iq, qc) for qc in range(4))
                avw = av_q.pop(0) if len(av_q) >= 2 else None
                filler = 8 * h if iq == 0 and h < 2 else None
                po = scores_block(h, qlo, u, avw=avw, filler=filler)
                if pending_out and h == 3:
                    while pending_out:
                        outproj_qc(*pending_out.pop(0))
                if po is not None:
                    fin = av_rec(avw[0], avw[1], po)
                    fin_h3, fin_iq = (avw[0] == hpg - 1), avw[1] // qt
                ew_block(h, u, mblk)
                av_q.append((h, qlo, u))
        # tail: drain the two-deep pipeline
        if fin is not None:
            av_fin(*fin)
        for (hh, qq, uu) in av_q:
            _, _, po = av_mm(hh, qq, uu)
            f = av_rec(hh, qq, po)
            av_fin(*f)
        pending_out.extend((NQ - 1, qc) for qc in range(4))
        for w in pending_out:
            outproj_qc(*w)

    nc.compile()
    return nc


def _shard_inputs(iQ, iK, mask, Wq, Wkv, Wo, nbias):
    in_maps = []
    maskT_by_b = [np.ascontiguousarray((~mask[b]).T).astype(BF16)
                  for b in range(B)]
    qT_by_b = [np.ascontiguousarray(iQ[b].T).astype(BF16) for b in range(B)]
    kT_by_b = [np.ascontiguousarray(iK[b].T).astype(BF16) for b in range(B)]
    nb = np.asarray(nbias, np.float32).reshape(1, 1)
    for ci in range(N_CORES):
        b, g = ci // GROUPS, ci % GROUPS
        hsl = slice(g * HS, (g + 1) * HS)
        in_maps.append({
            "qT": qT_by_b[b],
            "kT": kT_by_b[b],
            "wqT": np.ascontiguousarray(Wq[hsl].T).astype(BF16),
            "wkT": np.ascontiguousarray(Wkv[hsl].T).astype(BF16),
            "wvT": np.ascontiguousarray(Wkv[HSIZE + g * HS:HSIZE + (g + 1) * HS].T).astype(BF16),
            "woT": np.ascontiguousarray(Wo[:, hsl].T).astype(BF16),
            "maskT": maskT_by_b[b],
            "nbias": nb,
        })
    return in_maps


def kernel(iQ, iK, mask, Wq, Wkv, Wo, nbias):
    global _COMPILED
    from concourse.bass_utils import run_bass_kernel_spmd

    if _COMPILED is None:
        _COMPILED = _build()
    in_maps = _shard_inputs(np.asarray(iQ, np.float32), np.asarray(iK, np.float32),
                            np.asarray(mask), np.asarray(Wq, np.float32),
                            np.asarray(Wkv, np.float32), np.asarray(Wo, np.float32),
                            np.asarray(nbias, np.float32))
    res = run_bass_kernel_spmd(_COMPILED, in_maps, list(range(N_CORES))).results
    out = np.zeros((B, Q, D), np.float32)
    for ci in range(N_CORES):
        out[ci // GROUPS] += np.asarray(res[ci]["out"], np.float32)
    return out


# revision 49
# speedup vs baseline: 1.4167x; 1.0072x over previous
"""Sparse cross-attention (squared-ReLU normalizer) on 8 TRN2 NeuronCores.

Sharding: 8 cores = batch(2) x head-group(4). Each core owns one batch and
4 of 16 heads (a 256-wide slice of hsize): Wq/Wkv column-parallel,
Wo row-parallel (partial outputs summed on host), mask replicated per
batch shard.

Per-core kernel, engine-balanced (bf16 matmuls, fp32 PSUM):
  stage A: rqT (hs,q), rkT (hs,s) via weight-stationary projections into
    [128,1024] PSUM tiles; rv (s, hs+ones) with kT chunks as weights.
    rq evicted on ACT (scale 1/sqrt(a) folded), rk/rv evicted on DVE.
    Input DMA issues are spread across engine queues (weights on DVE,
    mask on ACT, x on sync) so transfers overlap from t=0.
  middle, per unit = (q-tile 512, head): AV matmuls of the previous head
    run first (their t is ready - a contiguous PE burst), then one out-
    projection chunk, then 8 score-matmul pairs -> [128,1024] PSUM, relu
    +nbias on ACT -> u; square IN PLACE (u*=u) and mask mul (t=u*maskT)
    on DVE with the tail s-chunks on GpSimd. AV uses rv chunks as
    weights giving oT (64,q) + denominator row; 1/den via ACT exp(-ln);
    GpSimd partition-broadcast spreads rec; DVE normalizes+evicts oT.
  out projection: oT^T @ woT in PSUM, evicted bf16 (ACT/DVE alternating),
    DMA'd to DRAM; host sums the 4 row-parallel partials in fp32.
"""

import numpy as np
import ml_dtypes

BF16 = ml_dtypes.bfloat16

B, Q, S, D = 2, 2048, 2048, 1024
NUM_HEAD, ADIM = 16, 64
HSIZE = NUM_HEAD * ADIM
N_CORES = 8
GROUPS = 4                  # head groups (tensor-parallel dim)
HPG = NUM_HEAD // GROUPS    # 4 heads per core
HS = HPG * ADIM             # 256: per-core hsize slice
P = 128

POOL_SC = 3                 # trailing s-chunks per mul pass on GpSimd

_COMPILED = None


def _build(q=Q, s=S, d=D, hpg=HPG, adim=ADIM, qt=512):
    """Build + compile the per-core Bass program. Returns the Bacc."""
    from contextlib import ExitStack
    import concourse.bass as bass
    import concourse.mybir as mybir
    import concourse.tile as tile
    from concourse import bacc

    fp32 = mybir.dt.float32
    bf16 = mybir.dt.bfloat16
    AF = mybir.ActivationFunctionType

    hs = hpg * adim
    DC = d // P          # contraction chunks for projections (8)
    NQ = q // qt         # q tiles (4)
    SC = s // P          # s chunks (16)
    HC = hs // P         # hsize-slice chunks (2)
    DVE_SC = SC - POOL_SC
    assert hs % P == 0 and q % qt == 0 and qt == 512 and d == 1024

    nc = bacc.Bacc("TRN2", target_bir_lowering=False, debug=False,
                   num_devices=N_CORES)

    qT = nc.dram_tensor("qT", [d, q], bf16, kind="ExternalInput").ap()
    kT = nc.dram_tensor("kT", [d, s], bf16, kind="ExternalInput").ap()
    wqT = nc.dram_tensor("wqT", [d, hs], bf16, kind="ExternalInput").ap()
    wkT = nc.dram_tensor("wkT", [d, hs], bf16, kind="ExternalInput").ap()
    wvT = nc.dram_tensor("wvT", [d, hs], bf16, kind="ExternalInput").ap()
    woT = nc.dram_tensor("woT", [hs, d], bf16, kind="ExternalInput").ap()
    maskT = nc.dram_tensor("maskT", [s, q], bf16, kind="ExternalInput").ap()
    nbias = nc.dram_tensor("nbias", [1, 1], fp32, kind="ExternalInput").ap()
    out = nc.dram_tensor("out", [q, d], bf16, kind="ExternalOutput").ap()

    qT_t = qT.rearrange("(c p) q -> c p q", p=P)        # [8, 128, q]
    kT_t = kT.rearrange("(c p) s -> c p s", p=P)
    wqT_t = wqT.rearrange("(c p) h -> c p h", p=P)
    wkT_t = wkT.rearrange("(c p) h -> c p h", p=P)
    wvT_t = wvT.rearrange("(c p) h -> c p h", p=P)
    woT_t = woT.rearrange("(c p) d -> c p d", p=P)      # [2, 128, d]
    maskT_t = maskT.rearrange("(c p) q -> p c q", p=P)  # [128, SC, q]
    out_t = out.rearrange("(c p) d -> c p d", p=P)      # [q/P, 128, d]

    scale = 1.0 / np.sqrt(np.float32(adim))

    with tile.TileContext(nc) as tc, ExitStack() as ctx:
        const = ctx.enter_context(tc.tile_pool(name="const", bufs=1))
        wpool = ctx.enter_context(tc.tile_pool(name="w", bufs=1))
        xpool = ctx.enter_context(tc.tile_pool(name="x", bufs=9))
        actp = ctx.enter_context(tc.tile_pool(name="act", bufs=1))
        maskp = ctx.enter_context(tc.tile_pool(name="mask", bufs=2))
        upool = ctx.enter_context(tc.tile_pool(name="u", bufs=3))
        t2p = ctx.enter_context(tc.tile_pool(name="t2", bufs=3))
        recp = ctx.enter_context(tc.tile_pool(name="rec", bufs=3))
        rbp = ctx.enter_context(tc.tile_pool(name="rb", bufs=2))
        outp = ctx.enter_context(tc.tile_pool(name="out", bufs=2))
        psB = ctx.enter_context(tc.tile_pool(name="psB", bufs=3, space="PSUM"))
        psS = ctx.enter_context(tc.tile_pool(name="psS", bufs=2, space="PSUM"))

        # ---- constants ----
        ones1 = const.tile([1, P], bf16)
        nc.any.memset(ones1[:], 1.0)
        ones1f = const.tile([1, P], fp32)
        nc.any.memset(ones1f[:], 1.0)
        nb1 = const.tile([1, 1], fp32)
        nc.sync.dma_start(nb1[:], nbias[:])
        ps_nb = psS.tile([P, 512], fp32, tag="ps", name="psnb")
        nc.tensor.matmul(ps_nb[:, 0:1], ones1f[:], nb1[:], start=True, stop=True)
        nb128 = const.tile([P, 1], fp32)
        nc.scalar.copy(nb128[:], ps_nb[:, 0:1])

        # ---- resident weights on the ACT DMA queue so the sync queue is
        # free for the x chunks from t=0 ----
        wq_sb = wpool.tile([P, DC, hs], bf16)
        wk_sb = wpool.tile([P, DC, hs], bf16)
        wv_sb = wpool.tile([P, DC, hs], bf16)
        wo_sb = wpool.tile([P, HC, d], bf16)
        # ---- resident activations ----
        rqT_sb = actp.tile([P, HC, q], bf16)    # (hs, q)
        # rk zero-padded to K=128 per head: scores matmuls then use the full
        # PE array (K=64 row-group mode costs ~1.6x per row); the rhs is the
        # full two-head rqT chunk - the other head's rows hit zero weights.
        rkT_sb = actp.tile([P, hpg, s], bf16)   # (128, head, s)
        rv_sb = actp.tile([P, SC, hpg * (adim + 1)], bf16)  # (s, hs + ones)
        oT_sb = actp.tile([P, HC, q], bf16)     # (hs, q)
        nc.gpsimd.memset(rv_sb[:], 1.0)         # ones cols survive at 64::65
        nc.gpsimd.memset(rkT_sb[:], 0.0)        # zero rows for the K pad

        # ---- stage A inputs; mask for the first q tile is hoisted before
        # the K-side loads; xk reuses xq buffers so its DMAs stream in as
        # Qm1 frees them chunk by chunk ----
        xq = []
        for c in range(DC):
            nc.sync.dma_start(wq_sb[:, c], wqT_t[c])
            xt = xpool.tile([P, q], bf16, tag="x", name=f"xq{c}")
            nc.sync.dma_start(xt[:], qT_t[c])
            xq.append(xt)
        for c in range(DC):
            nc.sync.dma_start(wk_sb[:, c], wkT_t[c])
            nc.sync.dma_start(wv_sb[:, c], wvT_t[c])
        xk = []
        for c in range(DC):
            xt = xpool.tile([P, s], bf16, tag="x", name=f"xk{c}")
            nc.sync.dma_start(xt[:], kT_t[c])
            xk.append(xt)
        mblk0 = maskp.tile([P, SC, qt], bf16, tag="m", name="mb0")
        nc.sync.dma_start(mblk0[:], maskT_t[:, :, 0:qt])
        for c in range(HC):
            nc.sync.dma_start(wo_sb[:, c], woT_t[c])

        def proj_block(m, w_sb, x_tiles, out_sb, on_act, sc_=1.0, pad=False):
            """out_sb[:, m, :] = (W_m @ X); matmul N capped at one PSUM bank.
            pad=True scatters the two head-halves into the K=128-padded rk
            layout (row half (h%2)*64 of plane h)."""
            for t2i in range(q // 1024):
                ps = psB.tile([P, 1024], fp32, tag="pb", name="pproj")
                for c in range(DC):
                    for j in (0, 1):
                        lo = t2i * 1024 + j * 512
                        nc.tensor.matmul(
                            ps[:, j * 512:(j + 1) * 512],
                            w_sb[:, c, m * P:(m + 1) * P],
                            x_tiles[c][:, lo:lo + 512],
                            start=(c == 0), stop=(c == DC - 1))
                span = slice(t2i * 1024, (t2i + 1) * 1024)
                if pad:
                    nc.vector.tensor_copy(out_sb[0:adim, 2 * m, span],
                                          ps[0:adim, :])
                    nc.vector.tensor_copy(out_sb[adim:P, 2 * m + 1, span],
                                          ps[adim:P, :])
                elif on_act:
                    nc.scalar.activation(out_sb[:, m, span], ps[:],
                                         AF.Copy, scale=float(sc_))
                else:
                    nc.vector.tensor_copy(out_sb[:, m, span], ps[:])

        def rv_block():
            for sc in range(SC):
                ps = psS.tile([P, 512], fp32, tag="ps", name="prv")
                for c in range(DC):
                    nc.tensor.matmul(
                        ps[:, :hs], xk[c][:, sc * P:(sc + 1) * P],
                        wv_sb[:, c], start=(c == 0), stop=(c == DC - 1))
                nc.vector.tensor_copy(
                    rv_sb[:, sc].rearrange("p (h c) -> p h c", c=adim + 1)[:, :, 0:adim],
                    ps[:, :hs].rearrange("p (h c) -> p h c", c=adim))

        # ---- middle-phase blocks ----
        # AV chunks from TWO units back are interleaved between score pairs:
        # their t is fully computed, so they never stall the in-order PE and
        # fill the relu-gated gaps between pairs (keeps the clock ramped).
        # They start at pair 2 so the po PSUM slot (freed by the previous
        # unit's oTnorm on DVE) is available by then.
        AV_SCHED = {2: (0, 4), 3: (4, 8), 4: (8, 12), 5: (12, 16)}

        def rv_part(lo, hi):
            """rv projection for s-chunks [lo,hi) - filler PE work emitted
            into the AV slots of the first two units (no AV work there)."""
            for sc in range(lo, hi):
                ps = psS.tile([P, 512], fp32, tag="ps", name="prv")
                for c in range(DC):
                    nc.tensor.matmul(
                        ps[:, :hs], xk[c][:, sc * P:(sc + 1) * P],
                        wv_sb[:, c], start=(c == 0), stop=(c == DC - 1))
                nc.vector.tensor_copy(
                    rv_sb[:, sc].rearrange("p (h c) -> p h c", c=adim + 1)[:, :, 0:adim],
                    ps[:, :hs].rearrange("p (h c) -> p h c", c=adim))

        def scores_block(h, qlo, u, avw=None, filler=None):
            hp, hc = (h % 2) * adim, h // 2
            po = None
            n_av = 0
            if avw is not None:
                hprev, qloprev, tprev = avw
                po = psS.tile([P, 512], fp32, tag="ps", name="pav")
            for k in range(8):          # sc pairs
                if po is not None and k in AV_SCHED:
                    lo, hi = AV_SCHED[k]
                    for sc in range(lo, hi):
                        nc.tensor.matmul(
                            po[0:adim + 1, :],
                            rv_sb[:, sc, hprev * (adim + 1):(hprev + 1) * (adim + 1)],
                            tprev[:, sc], start=(n_av == 0), stop=(n_av == SC - 1))
                        n_av += 1
                elif filler is not None and k in AV_SCHED:
                    base = filler + 2 * (k - 2)
                    rv_part(base, base + 2)
                ps = psB.tile([P, 1024], fp32, tag="pb", name="pscore")
                for j in (0, 1):
                    sc = 2 * k + j
                    nc.tensor.matmul(
                        ps[:, j * 512:(j + 1) * 512],
                        rkT_sb[:, h, sc * P:(sc + 1) * P],
                        rqT_sb[:, hc, qlo:qlo + qt],
                        start=True, stop=True)
                up = u[:, 2 * k:2 * k + 2].rearrange("p a b -> p (a b)")
                nc.scalar.activation(up, ps[:], AF.Relu, bias=nb128[:])
            return po

        def ew_block(h, u, mblk):
            """t = u^2 * mask via a scratch chunk: in-place DVE ops hit a
            ~4x slow path, so square writes t2 and the mask mul writes the
            final t back into u (both clean). u then feeds the AV matmuls
            two units later. GpSimd takes the last chunk."""
            for j in range(4):
                u4 = u[:, 4 * j:4 * j + 4].rearrange("p a b -> p (a b)")
                m4 = mblk[:, 4 * j:4 * j + 4].rearrange("p a b -> p (a b)")
                t2 = t2p.tile([P, 2048], bf16, tag="t2", name="t2t")
                eng = nc.gpsimd if j == 3 else nc.vector
                eng.tensor_mul(t2[:], u4, u4)
                eng.tensor_mul(u4, t2[:], m4)

        def av_mm(h, qlo, t):
            po = psS.tile([P, 512], fp32, tag="ps", name="pav")
            for sc in range(SC):
                nc.tensor.matmul(
                    po[0:adim + 1, :],
                    rv_sb[:, sc, h * (adim + 1):(h + 1) * (adim + 1)],
                    t[:, sc], start=(sc == 0), stop=(sc == SC - 1))
            return (h, qlo, po)

        def av_rec(h, qlo, po):
            """rec = 1/den on the ACT Reciprocal unit, via a raw instruction
            (the bass wrapper refuses it; its accuracy is amply within our
            2e-2 budget, and it shares an act table with Relu/Copy). The DVE
            reciprocal costs 3.4us per [1,512] row - far too slow."""
            rec = recp.tile([1, 512], bf16, tag="rec", name="rect")
            se = nc.scalar
            ins_l = [se.lower_ap(po[adim:adim + 1, :])]
            for arg in (0.0, 1.0, 0.0):     # bias, scale, alpha
                ins_l.append(mybir.ImmediateValue(dtype=fp32, value=arg))
            se.add_instruction(mybir.InstActivation(
                name=se.bass.get_next_instruction_name(),
                func=AF.Reciprocal, ins=ins_l, outs=[se.lower_ap(rec[:])]))
            return (h, qlo, po, rec)

        def av_fin(h, qlo, po, rec):
            """Broadcast rec via a K=1 PE outer product (GpSimd
            partition_broadcast forces Q7 library swaps costing ~7us/unit),
            copy it to SBUF (DVE reads at most one PSUM operand), then
            normalize + evict oT."""
            hp, hc = (h % 2) * adim, h // 2
            rb = psB.tile([P, 512], fp32, tag="pb", name="prb")
            nc.tensor.matmul(rb[0:adim, :], ones1[0:1, 0:adim], rec[:],
                             start=True, stop=True)
            rb_sb = rbp.tile([adim, 512], fp32, tag="rb", name="rbt")
            nc.vector.tensor_copy(rb_sb[:], rb[0:adim, :])
            nc.vector.tensor_mul(oT_sb[hp:hp + adim, hc, qlo:qlo + qt],
                                 po[0:adim, :], rb_sb[:])

        def outproj_qc(iq, qc):
            qlo = iq * qt
            ps = psB.tile([P, 1024], fp32, tag="pb", name="pout")
            for c in range(HC):
                for j in (0, 1):
                    nc.tensor.matmul(
                        ps[:, j * 512:(j + 1) * 512],
                        oT_sb[:, c, qlo + qc * P:qlo + (qc + 1) * P],
                        wo_sb[:, c, j * 512:(j + 1) * 512],
                        start=(c == 0), stop=(c == HC - 1))
            ob = outp.tile([P, 1024], bf16, tag="ob", name="obt")
            if qc % 2 == 0:
                nc.scalar.copy(ob[:], ps[:])
            else:
                nc.vector.tensor_copy(ob[:], ps[:])
            nc.sync.dma_start(out_t[iq * (qt // P) + qc], ob[:])

        # ---- stage A: all four projections back to back (continuous PE
        # ramp); the rv projection is deferred into the first two units'
        # empty AV slots so the middle phase starts ~14us earlier ----
        proj_block(0, wq_sb, xq, rqT_sb, on_act=True, sc_=scale)
        proj_block(1, wq_sb, xq, rqT_sb, on_act=True, sc_=scale)
        proj_block(0, wk_sb, xk, rkT_sb, on_act=False, pad=True)
        proj_block(1, wk_sb, xk, rkT_sb, on_act=False, pad=True)

        # ---- middle: units two-deep pipelined. Unit (iq,h): av_fin of the
        # unit three back; AV matmuls of the unit two back interleaved with
        # this unit's score pairs; reciprocal of the popped unit; in-place
        # elementwise of this unit. Out-projection of q tile iq-1 flushed at
        # h==3 (after its last av_fin). ----
        av_q = []             # (h, qlo, u) units awaiting AV matmuls
        fin = None            # av_rec result awaiting av_fin
        fin_h3, fin_iq = False, 0
        pending_out = []      # (iq, qc) out-projection chunks ready to emit
        for iq in range(NQ):
            qlo = iq * qt
            if iq == 0:
                mblk = mblk0
            else:
                mblk = maskp.tile([P, SC, qt], bf16, tag="m", name=f"mb{iq}")
                nc.sync.dma_start(mblk[:], maskT_t[:, :, qlo:qlo + qt])
            for h in range(hpg):
                u = upool.tile([P, SC, qt], bf16, tag="u", name="ut")
                if fin is not None:
                    av_fin(*fin)
                    fin = None
                    if fin_h3:
                        pending_out.extend((fin_iq, qc) for qc in range(4))
                avw = av_q.pop(0) if len(av_q) >= 2 else None
                filler = 8 * h if iq == 0 and h < 2 else None
                po = scores_block(h, qlo, u, avw=avw, filler=filler)
                if pending_out and h == 3:
                    while pending_out:
                        outproj_qc(*pending_out.pop(0))
                if po is not None:
                    fin = av_rec(avw[0], avw[1], po)
                    fin_h3, fin_iq = (avw[0] == hpg - 1), avw[1] // qt
                ew_block(h, u, mblk)
                av_q.append((h, qlo, u))
        # tail: drain the two-deep pipeline
        if fin is not None:
            av_fin(*fin)
        for (hh, qq, uu) in av_q:
            _, _, po = av_mm(hh, qq, uu)
            f = av_rec(hh, qq, po)
            av_fin(*f)
        pending_out.extend((NQ - 1, qc) for qc in range(4))
        for w in pending_out:
            outproj_qc(*w)

    nc.compile()
    return nc


def _shard_inputs(iQ, iK, mask, Wq, Wkv, Wo, nbias):
    in_maps = []
    maskT_by_b = [np.ascontiguousarray((~mask[b]).T).astype(BF16)
                  for b in range(B)]
    qT_by_b = [np.ascontiguousarray(iQ[b].T).astype(BF16) for b in range(B)]
    kT_by_b = [np.ascontiguousarray(iK[b].T).astype(BF16) for b in range(B)]
    nb = np.asarray(nbias, np.float32).reshape(1, 1)
    for ci in range(N_CORES):
        b, g = ci // GROUPS, ci % GROUPS
        hsl = slice(g * HS, (g + 1) * HS)
        in_maps.append({
            "qT": qT_by_b[b],
            "kT": kT_by_b[b],
            "wqT": np.ascontiguousarray(Wq[hsl].T).astype(BF16),
            "wkT": np.ascontiguousarray(Wkv[hsl].T).astype(BF16),
            "wvT": np.ascontiguousarray(Wkv[HSIZE + g * HS:HSIZE + (g + 1) * HS].T).astype(BF16),
            "woT": np.ascontiguousarray(Wo[:, hsl].T).astype(BF16),
            "maskT": maskT_by_b[b],
            "nbias": nb,
        })
    return in_maps


def kernel(iQ, iK, mask, Wq, Wkv, Wo, nbias):
    global _COMPILED
    from concourse.bass_utils import run_bass_kernel_spmd

    if _COMPILED is None:
        _COMPILED = _build()
    in_maps = _shard_inputs(np.asarray(iQ, np.float32), np.asarray(iK, np.float32),
                            np.asarray(mask), np.asarray(Wq, np.float32),
                            np.asarray(Wkv, np.float32), np.asarray(Wo, np.float32),
                            np.asarray(nbias, np.float32))
    res = run_bass_kernel_spmd(_COMPILED, in_maps, list(range(N_CORES))).results
    out = np.zeros((B, Q, D), np.float32)
    for ci in range(N_CORES):
        out[ci // GROUPS] += np.asarray(res[ci]["out"], np.float32)
    return out
